# revision 1
# baseline (speedup 1.0000x reference)
"""Trainium2 Bass kernel for single-head cross-attention with additive mask.

Computation (matches the reference):
    q = tgt @ wq + bq
    k = src @ wk (+ bk dropped: softmax cancels a per-row constant exactly)
    v = src @ wv (bv folded into the epilogue: out = attn@v + bv)
    s = (q k^T + mask) / sqrt(DQ)
    out = softmax(s) @ v + bv

Two SPMD launches on 8 cores:
  L1: each core projects kT (fp32 psum) and v (fp16 matmuls) for 1/8 of the
      global (B*S) src rows from a host-pre-transposed src slice.
  host: concatenates the 8 K/V shards, appends the softmax-denominator ones
      column to V, casts K/V to fp16 (pure layout glue, no math).
  L2: tgt sharded 8 ways; core c handles tgt rows [c*512,(c+1)*512) of every
      batch so its 8MB mask slice is read from HBM exactly once.

Scores are built transposed (src rows on PSUM partitions) so the PV matmul
consumes softmax weights directly, batch-pair outer so the QK psum tile can
triple-buffer. Projections accumulate in fp32; Q/K/V/mask/P are fp16 (11
mantissa bits keeps rel-err ~1e-3 at full matmul speed). The mask is added
by the otherwise-idle DVE (fp16-cast on the fly by the load DMA), exp() runs
on ACT and emits fp16 attention weights, PV accumulates fp32 in PSUM, and
the epilogue (1/rowsum scaling + bv bias) is PE-free via gpsimd
partition_broadcast. The output leaves transposed [B, DQ, TS]; the host
flips it.
"""
import numpy as np

B, S, D, DQ = 4, 4096, 1024, 64
NCORES = 8
TS = S // NCORES            # 512 tgt rows per core
SR = (B * S) // NCORES      # 2048 global src rows per core (L1)
SB = S // 128               # 32 src blocks per batch
GK = B * SB                 # 128 global src blocks
CORES = list(range(NCORES))
F32 = np.float32
FP16 = np.float16

_CACHE = {}


def _build_l1():
    import concourse.mybir as mybir
    import concourse.tile as tile
    from concourse import bacc

    f32 = mybir.dt.float32
    fp16 = mybir.dt.float16

    nc = bacc.Bacc("TRN2", target_bir_lowering=False, debug=False,
                   num_devices=NCORES)
    srcT = nc.dram_tensor("srcT", [D, SR], f32, kind="ExternalInput")
    wk = nc.dram_tensor("wk", [D, DQ], f32, kind="ExternalInput")
    wv = nc.dram_tensor("wv", [D, DQ], f32, kind="ExternalInput")
    kt = nc.dram_tensor("kt", [DQ, 2, 1024], f32, kind="ExternalOutput")
    vout = nc.dram_tensor("vout", [SR, DQ], f32, kind="ExternalOutput")

    with tile.TileContext(nc) as tc:
        with (
            tc.tile_pool(name="const", bufs=1) as constp,
            tc.tile_pool(name="big", bufs=1) as bigp,
            tc.tile_pool(name="stream", bufs=2) as streamp,
            tc.tile_pool(name="pp", bufs=1, space="PSUM") as pp,
        ):
            wk_sb = constp.tile([128, 8 * DQ], f32)
            nc.sync.dma_start(
                out=wk_sb.rearrange("p (j m) -> p j m", m=DQ),
                in_=wk.rearrange("(j p) m -> p j m", p=128))
            wv_bf = constp.tile([128, 8 * DQ], fp16)
            nc.gpsimd.dma_start(
                out=wv_bf.rearrange("p (j m) -> p j m", m=DQ),
                in_=wv.rearrange("(j p) m -> p j m", p=128))

            kT_psA = pp.tile([128, 1024], f32, tag="qk0")
            kT_psB = pp.tile([128, 1024], f32, tag="qk1")
            v_ps = [pp.tile([128, 4 * DQ], f32, tag=f"pv{q}", name=f"v_ps{q}")
                    for q in range(4)]
            for j in range(8):
                st = streamp.tile([128, SR], f32, tag="xs", bufs=3)
                nc.sync.dma_start(out=st[:], in_=srcT[j * 128:(j + 1) * 128, :])
                stb = streamp.tile([128, SR], fp16, tag="xsb")
                nc.vector.tensor_copy(stb[:], st[:])
                for g in (0, 2, 1, 3):  # alternate col-groups for PE overlap
                    if g < 2:
                        ps, col, tp, po = kT_psA, g * 512, (0, 0), 0
                    else:
                        ps, col, tp, po = kT_psB, (g - 2) * 512, (0, 64), 64
                    nc.tensor.matmul(
                        ps[po:po + 64, col:col + 512],
                        lhsT=wk_sb[:, j * DQ:(j + 1) * DQ],
                        rhs=st[:, g * 512:(g + 1) * 512],
                        start=(j == 0), stop=(j == 7), tile_position=tp)
                for k in range(16):
                    nc.tensor.matmul(
                        v_ps[k // 4][:, (k % 4) * DQ:(k % 4 + 1) * DQ],
                        lhsT=stb[:, k * 128:(k + 1) * 128],
                        rhs=wv_bf[:, j * DQ:(j + 1) * DQ],
                        start=(j == 0 and k % 4 == 0),
                        stop=(j == 7 and k % 4 == 3))
            kT_sb = bigp.tile([128, 1024], f32)
            nc.scalar.copy(kT_sb[0:64, :], kT_psA[0:64, :])
            nc.scalar.copy(kT_sb[64:128, :], kT_psB[64:128, :])
            v_sb = bigp.tile([128, 16 * DQ], f32)
            for q in range(4):
                nc.vector.tensor_copy(v_sb[:, q * 256:(q + 1) * 256], v_ps[q][:])
            nc.sync.dma_start(out=kt[:, 0, :], in_=kT_sb[0:64, :])
            nc.sync.dma_start(out=kt[:, 1, :], in_=kT_sb[64:128, :])
            nc.gpsimd.dma_start(
                out=vout.rearrange("(k p) d -> p k d", p=128),
                in_=v_sb.rearrange("p (k d) -> p k d", d=DQ))
    nc.compile()
    return nc


def _build_l2():
    import concourse.mybir as mybir
    import concourse.tile as tile
    from concourse import bacc
    from concourse.masks import make_identity

    f32 = mybir.dt.float32
    fp16 = mybir.dt.float16
    AF = mybir.ActivationFunctionType

    nc = bacc.Bacc("TRN2", target_bir_lowering=False, debug=False,
                   num_devices=NCORES)
    # kT2 layout: partitions 0-63 = d, s of batches 0-1; 64-127 = batches 2-3
    kt2d = nc.dram_tensor("kt2", [128, 2 * S], fp16, kind="ExternalInput")
    # v65 in SBUF layout: row p, cols (k, c): element = v[k*128 + p, c] | ones
    v65d = nc.dram_tensor("v65", [128, GK * (DQ + 1)], fp16, kind="ExternalInput")
    tgtT = nc.dram_tensor("tgtT", [B, D, TS], f32, kind="ExternalInput")
    # host-transposed mask slice: masknT[s, t] = mask[c*TS + t, s]
    masknT = nc.dram_tensor("masknT", [S, TS], f32, kind="ExternalInput")
    wq = nc.dram_tensor("wq", [D, DQ], f32, kind="ExternalInput")
    bq = nc.dram_tensor("bq", [DQ], f32, kind="ExternalInput")
    bv = nc.dram_tensor("bv", [DQ], f32, kind="ExternalInput")
    # transposed output: host flips [B, DQ, TS] -> [B, TS, DQ]
    out = nc.dram_tensor("out", [B, DQ, TS], f32, kind="ExternalOutput")

    with tile.TileContext(nc) as tc:
        with (
            tc.tile_pool(name="const", bufs=1) as constp,
            tc.tile_pool(name="big", bufs=1) as bigp,
            tc.tile_pool(name="stream", bufs=2) as streamp,
            tc.tile_pool(name="pp", bufs=1, space="PSUM") as pp,
        ):
            wq_sb = constp.tile([128, 8 * DQ], f32)
            nc.sync.dma_start(
                out=wq_sb.rearrange("p (j m) -> p j m", m=DQ),
                in_=wq.rearrange("(j p) m -> p j m", p=128))
            bq_sb = constp.tile([128, 1], f32)
            nc.sync.dma_start(out=bq_sb[0:64, :], in_=bq.rearrange("(p o) -> p o", o=1))
            nc.sync.dma_start(out=bq_sb[64:128, :], in_=bq.rearrange("(p o) -> p o", o=1))
            bv_sb = constp.tile([64, 1], f32)
            nc.sync.dma_start(out=bv_sb[:], in_=bv.rearrange("(p o) -> p o", o=1))

            # resident loads, chunked so sg=0 unblocks early
            kT2 = bigp.tile([128, 2 * S], fp16)
            for q4 in (0, 2, 1, 3):  # first halves of both batch-halves first
                nc.sync.dma_start(out=kT2[:, q4 * 2048:(q4 + 1) * 2048],
                                  in_=kt2d[:, q4 * 2048:(q4 + 1) * 2048])
            v2 = bigp.tile([128, GK * (DQ + 1)], fp16)
            VQ = 32 * (DQ + 1)
            for q4 in range(4):
                nc.gpsimd.dma_start(out=v2[:, q4 * VQ:(q4 + 1) * VQ],
                                    in_=v65d[:, q4 * VQ:(q4 + 1) * VQ])
            # maskT, fp16-cast on the fly, duplicated per batch-half so one
            # [128, 1024] DVE add covers a whole score-pair tile:
            # layout [128 s-partitions, (sg, half, t)]
            maskTd = bigp.tile([128, SB * 2 * TS], fp16)
            mview = maskTd.rearrange("p (sb h t) -> p sb h t", h=2, t=TS)
            for g in range(4):
                nc.gpsimd.dma_start(
                    out=mview[:, g * 8:(g + 1) * 8, 0, :],
                    in_=masknT[g * 1024:(g + 1) * 1024, :]
                    .rearrange("(sb p) t -> p sb t", p=128))
                nc.vector.tensor_copy(mview[:, g * 8:(g + 1) * 8, 1, :],
                                      mview[:, g * 8:(g + 1) * 8, 0, :])

            # qT projection (fp32 matmuls, fp16 output for the fp16 QK)
            qT_sb = bigp.tile([128, 2 * TS], fp16)
            for b in range(B):
                pb, colb = (b // 2) * 64, (b % 2) * TS
                q_ps = pp.tile([128, TS], f32, tag="qk", bufs=3, name=f"q_ps{b}")
                for half in range(2):
                    tg = streamp.tile([128, SR], f32, tag="xs", bufs=3,
                                      name=f"tg{b}_{half}")
                    nc.sync.dma_start(
                        out=tg.rearrange("p (j t) -> p j t", t=TS),
                        in_=tgtT[b, half * 512:(half + 1) * 512, :]
                        .rearrange("(j p) t -> p j t", p=128))
                    for jj in range(4):
                        j = half * 4 + jj
                        nc.tensor.matmul(
                            q_ps[pb:pb + 64, :],
                            lhsT=wq_sb[:, j * DQ:(j + 1) * DQ],
                            rhs=tg[:, jj * TS:(jj + 1) * TS],
                            start=(j == 0), stop=(j == 7), tile_position=(0, pb))
                nc.scalar.activation(
                    qT_sb[pb:pb + 64, colb:colb + TS], q_ps[pb:pb + 64, :],
                    AF.Identity, bias=bq_sb[pb:pb + 64, :])

            # attention main loop: batch-pair outer so the QK psum tile can
            # triple-buffer (3 x 2 banks) against the DVE/ACT consumers.
            for pair in range(2):
                pb = pair * 64
                pv_ps = [pp.tile([65, TS], f32, tag=f"pv{h}",
                                 name=f"pv_ps{pair}_{h}") for h in range(2)]
                for sg in range(SB):
                    qkt = pp.tile([128, 2 * TS], f32, tag="qk", bufs=3,
                                  name=f"qkt{pair}_{sg}")
                    for half in range(2):
                        nc.tensor.matmul(
                            qkt[:, half * TS:(half + 1) * TS],
                            lhsT=kT2[pb:pb + 64, half * S + sg * 128:
                                     half * S + sg * 128 + 128],
                            rhs=qT_sb[pb:pb + 64, half * TS:(half + 1) * TS],
                            start=True, stop=True, tile_position=(pb, 0))
                    es = streamp.tile([128, 2 * TS], f32, tag="E", bufs=4,
                                      name=f"es{pair}_{sg}")
                    nc.vector.tensor_add(
                        es[:], qkt[:],
                        maskTd[:, sg * 2 * TS:(sg + 1) * 2 * TS])
                    pt = streamp.tile([128, 2 * TS], fp16, tag="P", bufs=6,
                                      name=f"pt{pair}_{sg}")
                    nc.scalar.activation(pt[:], es[:], AF.Exp, scale=0.125)
                    for half in range(2):
                        b = pair * 2 + half
                        kg = b * SB + sg
                        nc.tensor.matmul(
                            pv_ps[half][:],
                            lhsT=v2[:, kg * (DQ + 1):(kg + 1) * (DQ + 1)],
                            rhs=pt[:, half * TS:(half + 1) * TS],
                            start=(sg == 0), stop=(sg == SB - 1))

                # epilogue: out^T = pv[0:64]/sums + bv, all PE-free
                for half in range(2):
                    b = pair * 2 + half
                    sums = streamp.tile([65, TS], f32, tag="sums")
                    nc.scalar.copy(sums[64:65, :], pv_ps[half][64:65, :])
                    sums0 = streamp.tile([1, TS], f32, tag="sums0")
                    nc.sync.dma_start(out=sums0[:], in_=sums[64:65, :])
                    recip = streamp.tile([1, TS], f32, tag="recip")
                    rscr = streamp.tile([1, TS], f32, tag="rscr")
                    nc.vector.reciprocal_approx_accurate(recip[:], sums0[:],
                                                         rscr[:])
                    rb = streamp.tile([64, TS], f32, tag="rb")
                    nc.gpsimd.partition_broadcast(rb[:], recip[:])
                    ot = streamp.tile([64, TS], f32, tag="ot")
                    nc.vector.tensor_mul(ot[:], pv_ps[half][0:64, :], rb[:])
                    of = streamp.tile([64, TS], f32, tag="of")
                    nc.scalar.activation(of[:], ot[:], AF.Identity, bias=bv_sb[:])
                    nc.gpsimd.dma_start(out=out[b], in_=of[:])
    nc.compile()
    return nc


def _get_l1():
    if "l1" not in _CACHE:
        _CACHE["l1"] = _build_l1()
    return _CACHE["l1"]


def _get_l2():
    if "l2" not in _CACHE:
        _CACHE["l2"] = _build_l2()
    return _CACHE["l2"]


def make_in_maps_l1(src, wk, wv):
    src_flat = np.ascontiguousarray(src, dtype=F32).reshape(B * S, D)
    wk = np.ascontiguousarray(wk, dtype=F32)
    wv = np.ascontiguousarray(wv, dtype=F32)
    return [{
        "srcT": np.ascontiguousarray(src_flat[c * SR:(c + 1) * SR, :].T),
        "wk": wk, "wv": wv,
    } for c in CORES]


def glue_l1_outputs(results):
    """Assemble full kT2 / v65 arrays from the 8 per-core L1 outputs."""
    kts = [np.asarray(results[c]["kt"]).reshape(DQ, 2 * 1024) for c in CORES]
    kT_full = np.concatenate(kts, axis=1)            # [64, B*S]
    kt2 = np.concatenate([kT_full[:, :2 * S], kT_full[:, 2 * S:]],
                         axis=0).astype(FP16)
    v_full = np.concatenate(
        [np.asarray(results[c]["vout"]) for c in CORES], axis=0)  # [B*S, 64]
    v65 = np.empty((B * S, DQ + 1), dtype=FP16)
    v65[:, :DQ] = v_full.astype(FP16)
    v65[:, DQ] = np.asarray(1.0, dtype=FP16)
    # rearrange to the L2 SBUF layout: [128 partitions, (block k, col c)]
    v65 = np.ascontiguousarray(
        v65.reshape(GK, 128, DQ + 1).transpose(1, 0, 2).reshape(128, -1))
    return np.ascontiguousarray(kt2), v65


def make_in_maps_l2(kt2, v65, tgt, mask, wq, bq, bv):
    tgt = np.ascontiguousarray(tgt, dtype=F32)
    mask = np.ascontiguousarray(mask, dtype=F32)
    wq = np.ascontiguousarray(wq, dtype=F32)
    bq = np.ascontiguousarray(bq, dtype=F32)
    bv = np.ascontiguousarray(bv, dtype=F32)
    return [{
        "kt2": kt2, "v65": v65,
        "tgtT": np.ascontiguousarray(
            tgt[:, c * TS:(c + 1) * TS, :].transpose(0, 2, 1)),
        "masknT": np.ascontiguousarray(mask[c * TS:(c + 1) * TS, :].T),
        "wq": wq, "bq": bq, "bv": bv,
    } for c in CORES]


def kernel(src, tgt, mask, wq, bq, wk, bk, wv, bv):
    from concourse.bass_utils import run_bass_kernel_spmd

    res1 = run_bass_kernel_spmd(_get_l1(), make_in_maps_l1(src, wk, wv),
                                core_ids=CORES)
    kt2, v65 = glue_l1_outputs(res1.results)
    res2 = run_bass_kernel_spmd(
        _get_l2(), make_in_maps_l2(kt2, v65, tgt, mask, wq, bq, bv),
        core_ids=CORES)
    out = np.empty((B, S, DQ), dtype=F32)
    for c in CORES:
        out[:, c * TS:(c + 1) * TS, :] = \
            np.asarray(res2.results[c]["out"]).transpose(0, 2, 1)
    return out



# revision 2
# speedup vs baseline: 1.2640x; 1.2640x over previous
"""Trainium2 Bass kernel for single-head cross-attention with additive mask.

Computation (matches the reference):
    q = tgt @ wq + bq
    k = src @ wk (+ bk dropped: softmax cancels a per-row constant exactly)
    v = src @ wv (bv applied on host in the epilogue)
    s = (q k^T + mask) / sqrt(DQ)
    out = softmax(s) @ v + bv

Two SPMD launches on 8 cores (all matmul inputs fp16, fp32 PSUM accum):
  L1: each core projects K and V for 1/8 of the global (B*S) src rows in a
      single fused matmul (wk|wv concatenated -> kvT [128, 2048] fp16 out).
  host: pure layout glue -- assembles kt2 (d-major K) and v65 (V with an
      appended ones column for the softmax denominator).
  L2: tgt sharded 8 ways; core c handles tgt rows [c*512,(c+1)*512) of every
      batch so its mask slice is read from HBM exactly once.

L2 computes scores transposed (src-block on PSUM partitions) batch-column
outer: column c processes batches {c, c+2} whose QK matmuls use disjoint
64-row PE tiles (tile_position) and run concurrently.  The additive mask
enters PSUM ahead of QK via an identity-weight matmul (start=True), so the
scalar engine reads (qk+mask) straight from PSUM and emits fp16 exp at
scale=1/8.  A subset of src-blocks (DVE_SGS) instead computes exp on the
otherwise-idle vector engine with a Schraudolph bit-trick in the fp16 bit
domain: bits16 = int16(qk*A16 + bmask), where bmask (host-baked int16)
carries mask*A16 + (15-sigma)*1024; the int16 tile bitcast to fp16 IS the
approximate exp (rel err ~3%, diluted to ~1.6e-2 end-to-end).  PV
accumulates fp32 in PSUM with a 65th "ones" output row; the final
division by the softmax denominator (+bv) runs on the host.
"""
import numpy as np

B, S, D, DQ = 4, 4096, 1024, 64
NCORES = 8
TS = S // NCORES            # 512 tgt rows per core
SR = (B * S) // NCORES      # 2048 global src rows per core (L1)
SB = S // 128               # 32 src blocks per batch
GK = B * SB                 # 128 global src blocks
CORES = list(range(NCORES))
F32 = np.float32
FP16 = np.float16

# --- DVE fast-exp (Schraudolph in fp16 bit domain) ---
N_DVE_SG = 14               # how many of the 32 src-blocks use the DVE path
SIGMA = 0.0579
A16 = (2.0 ** 10) * np.log2(np.e) / 8.0
B16C = (2.0 ** 10) * (15.0 - SIGMA)
# spread the DVE blocks evenly through the sg loop so ACT/DVE interleave
DVE_SGS = tuple(g for g in range(SB)
                if (g + 1) * N_DVE_SG // SB > g * N_DVE_SG // SB)
ACT_SGS = tuple(g for g in range(SB) if g not in DVE_SGS)
N_ASG, N_DSG = len(ACT_SGS), len(DVE_SGS)
# position of each sg within its path's packed mask array
_MASK_IDX = {g: i for i, g in enumerate(ACT_SGS)}
_MASK_IDX.update({g: i for i, g in enumerate(DVE_SGS)})

_CACHE = {}


def _build_l1():
    import concourse.mybir as mybir
    import concourse.tile as tile
    from concourse import bacc

    f32 = mybir.dt.float32
    fp16 = mybir.dt.float16

    nc = bacc.Bacc("TRN2", target_bir_lowering=False, debug=False,
                   num_devices=NCORES)
    srcT = nc.dram_tensor("srcT", [D, SR], fp16, kind="ExternalInput")
    wkv = nc.dram_tensor("wkv", [D, 128], fp16, kind="ExternalInput")
    kvT = nc.dram_tensor("kvT", [128, SR], fp16, kind="ExternalOutput")

    with tile.TileContext(nc) as tc:
        with (
            tc.tile_pool(name="const", bufs=1) as constp,
            tc.tile_pool(name="big", bufs=1) as bigp,
            tc.tile_pool(name="stream", bufs=3) as streamp,
            tc.tile_pool(name="pp", bufs=1, space="PSUM") as pp,
        ):
            wkv_sb = constp.tile([128, 8 * 128], fp16)
            nc.sync.dma_start(
                out=wkv_sb.rearrange("p (j m) -> p j m", m=128),
                in_=wkv.rearrange("(j p) m -> p j m", p=128))
            kv_ps = pp.tile([128, SR], f32)
            for j in range(8):
                st = streamp.tile([128, SR], fp16, tag="xs", name=f"st{j}")
                nc.sync.dma_start(out=st[:], in_=srcT[j * 128:(j + 1) * 128, :])
                for q in range(4):
                    nc.tensor.matmul(
                        kv_ps[:, q * 512:(q + 1) * 512],
                        lhsT=wkv_sb[:, j * 128:(j + 1) * 128],
                        rhs=st[:, q * 512:(q + 1) * 512],
                        start=(j == 0), stop=(j == 7))
            kv_sb = bigp.tile([128, SR], fp16)
            nc.scalar.copy(kv_sb[:, 0:1024], kv_ps[:, 0:1024])
            nc.scalar.copy(kv_sb[:, 1024:2048], kv_ps[:, 1024:2048])
            nc.sync.dma_start(out=kvT[:], in_=kv_sb[:])
    nc.compile()
    return nc


def _build_l2():
    import concourse.mybir as mybir
    import concourse.tile as tile
    from concourse import bacc
    from concourse.masks import make_identity

    f32 = mybir.dt.float32
    fp16 = mybir.dt.float16
    i16 = mybir.dt.int16
    AF = mybir.ActivationFunctionType
    ALU = mybir.AluOpType

    nc = bacc.Bacc("TRN2", target_bir_lowering=False, debug=False,
                   num_devices=NCORES)
    # kt2: partitions 0-63 = kT of batches 0|1; 64-127 = batches 2|3
    kt2d = nc.dram_tensor("kt2", [128, 2 * S], fp16, kind="ExternalInput")
    # v65 layout: row p, cols (kg, c): element = v[kg*128 + p, c] | ones
    v65d = nc.dram_tensor("v65", [128, GK * (DQ + 1)], fp16,
                          kind="ExternalInput")
    tgtT = nc.dram_tensor("tgtT", [B, D, TS], fp16, kind="ExternalInput")
    # packed transposed mask rows for the ACT-path src blocks (fp16)
    if N_ASG:
        maskA = nc.dram_tensor("maskA", [N_ASG * 128, TS], fp16,
                               kind="ExternalInput")
    # packed Schraudolph bias rows for the DVE-path src blocks (int16)
    if N_DSG:
        bmaskd = nc.dram_tensor("bmask", [N_DSG * 128, TS], i16,
                                kind="ExternalInput")
    wq = nc.dram_tensor("wq", [D, DQ], fp16, kind="ExternalInput")
    bq = nc.dram_tensor("bq", [DQ], f32, kind="ExternalInput")
    # out rows 0-63: (attn @ v)^T numerator; row 64: softmax denominator
    o = nc.dram_tensor("o", [B, DQ + 1, TS], f32, kind="ExternalOutput")

    with tile.TileContext(nc) as tc:
        with (
            tc.tile_pool(name="const", bufs=1) as constp,
            tc.tile_pool(name="big", bufs=1) as bigp,
            tc.tile_pool(name="stream", bufs=2) as streamp,
            tc.tile_pool(name="pp", bufs=1, space="PSUM") as pp,
        ):
            wq_sb = constp.tile([128, 8 * DQ], fp16)
            nc.sync.dma_start(
                out=wq_sb.rearrange("p (j m) -> p j m", m=DQ),
                in_=wq.rearrange("(j p) m -> p j m", p=128))
            bq_sb = constp.tile([128, 1], f32)
            nc.sync.dma_start(out=bq_sb[0:64, :],
                              in_=bq.rearrange("(p o) -> p o", o=1))
            nc.sync.dma_start(out=bq_sb[64:128, :],
                              in_=bq.rearrange("(p o) -> p o", o=1))
            ident = constp.tile([128, 128], fp16)
            make_identity(nc, ident[:])

            # resident K/V, chunked in consumption order (column 0 first)
            kT2 = bigp.tile([128, 2 * S], fp16)
            for q4 in range(4):
                nc.sync.dma_start(out=kT2[:, q4 * 2048:(q4 + 1) * 2048],
                                  in_=kt2d[:, q4 * 2048:(q4 + 1) * 2048])
            v2 = bigp.tile([128, GK * (DQ + 1)], fp16)
            VQ = 32 * (DQ + 1)
            for bb in (0, 2, 1, 3):  # column order: b0, b2, then b1, b3
                nc.gpsimd.dma_start(out=v2[:, bb * VQ:(bb + 1) * VQ],
                                    in_=v65d[:, bb * VQ:(bb + 1) * VQ])
            # packed masks, sg-major, loaded in 4 chunks each
            if N_ASG:
                maskA_sb = bigp.tile([128, N_ASG * TS], fp16)
                mav = maskA_sb.rearrange("p (g t) -> p g t", t=TS)
                for lo in range(0, N_ASG, 8):
                    hi = min(lo + 8, N_ASG)
                    nc.sync.dma_start(
                        out=mav[:, lo:hi, :],
                        in_=maskA[lo * 128:hi * 128, :]
                        .rearrange("(g p) t -> p g t", p=128))
            if N_DSG:
                bmask_sb = bigp.tile([128, N_DSG * TS], i16)
                bmv = bmask_sb.rearrange("p (g t) -> p g t", t=TS)
                for lo in range(0, N_DSG, 8):
                    hi = min(lo + 8, N_DSG)
                    nc.gpsimd.dma_start(
                        out=bmv[:, lo:hi, :],
                        in_=bmaskd[lo * 128:hi * 128, :]
                        .rearrange("(g p) t -> p g t", p=128))

            # qT projection; column order so column 0's batches finish first
            qT_sb = bigp.tile([128, 2 * TS], fp16)
            for b in (0, 2, 1, 3):
                pb, colb = (b // 2) * 64, (b % 2) * TS
                q_ps = pp.tile([128, TS], f32, tag="qk", bufs=3,
                               name=f"q_ps{b}")
                for half in range(2):
                    tg = streamp.tile([128, SR], fp16, tag="xs", bufs=3,
                                      name=f"tg{b}_{half}")
                    nc.sync.dma_start(
                        out=tg.rearrange("p (j t) -> p j t", t=TS),
                        in_=tgtT[b, half * 512:(half + 1) * 512, :]
                        .rearrange("(j p) t -> p j t", p=128))
                    for jj in range(4):
                        j = half * 4 + jj
                        nc.tensor.matmul(
                            q_ps[pb:pb + 64, :],
                            lhsT=wq_sb[:, j * DQ:(j + 1) * DQ],
                            rhs=tg[:, jj * TS:(jj + 1) * TS],
                            start=(j == 0), stop=(j == 7),
                            tile_position=(0, pb))
                nc.scalar.activation(
                    qT_sb[pb:pb + 64, colb:colb + TS], q_ps[pb:pb + 64, :],
                    AF.Identity, bias=bq_sb[pb:pb + 64, :])

            # attention main loop: batch-column outer; column c handles
            # batches {c, c+2} on disjoint 64-row PE tiles.
            for col in range(2):
                pv = [pp.tile([DQ + 1, TS], f32, tag=f"pv{h}",
                              name=f"pv{col}_{h}") for h in range(2)]
                for sg in range(SB):
                    dve = sg in DVE_SGS
                    mi = _MASK_IDX[sg]
                    qkt = pp.tile([128, 2 * TS], f32, tag="qk", bufs=3,
                                  name=f"qkt{col}_{sg}")
                    if not dve:
                        # mask enters PSUM first via identity-weight matmul
                        for half in range(2):
                            nc.tensor.matmul(
                                qkt[:, half * TS:(half + 1) * TS],
                                lhsT=ident[:],
                                rhs=maskA_sb[:, mi * TS:(mi + 1) * TS],
                                start=True, stop=False)
                    for half in range(2):
                        bb = col + 2 * half
                        nc.tensor.matmul(
                            qkt[:, half * TS:(half + 1) * TS],
                            lhsT=kT2[half * 64:half * 64 + 64,
                                     col * S + sg * 128:
                                     col * S + sg * 128 + 128],
                            rhs=qT_sb[half * 64:half * 64 + 64,
                                      col * TS:(col + 1) * TS],
                            start=dve, stop=True,
                            tile_position=(half * 64, 0))
                    pt = streamp.tile([128, 2 * TS], fp16, tag="P", bufs=6,
                                      name=f"pt{col}_{sg}")
                    if not dve:
                        nc.scalar.activation(pt[:], qkt[:], AF.Exp,
                                             scale=0.125)
                    else:
                        for half in range(2):
                            nc.vector.scalar_tensor_tensor(
                                out=pt[:, half * TS:(half + 1) * TS]
                                .bitcast(i16),
                                in0=qkt[:, half * TS:(half + 1) * TS],
                                scalar=float(A16),
                                in1=bmask_sb[:, mi * TS:(mi + 1) * TS],
                                op0=ALU.mult, op1=ALU.add)
                    for half in range(2):
                        kg = (col + 2 * half) * SB + sg
                        nc.tensor.matmul(
                            pv[half][:],
                            lhsT=v2[:, kg * (DQ + 1):(kg + 1) * (DQ + 1)],
                            rhs=pt[:, half * TS:(half + 1) * TS],
                            start=(sg == 0), stop=(sg == SB - 1))

                for half in range(2):
                    ob = streamp.tile([DQ + 1, TS], f32, tag="ob", bufs=2,
                                      name=f"ob{col}_{half}")
                    nc.scalar.copy(ob[:], pv[half][:])
                    nc.gpsimd.dma_start(out=o[col + 2 * half], in_=ob[:])
    nc.compile()
    return nc


def _get_l1():
    if "l1" not in _CACHE:
        _CACHE["l1"] = _build_l1()
    return _CACHE["l1"]


def _get_l2():
    if "l2" not in _CACHE:
        _CACHE["l2"] = _build_l2()
    return _CACHE["l2"]


def make_in_maps_l1(src, wk, wv):
    src16 = np.asarray(src).astype(FP16).reshape(B * S, D)
    wkv = np.concatenate([np.asarray(wk), np.asarray(wv)],
                         axis=1).astype(FP16)
    return [{
        "srcT": np.ascontiguousarray(src16[c * SR:(c + 1) * SR, :].T),
        "wkv": wkv,
    } for c in CORES]


def glue_l1_outputs(results):
    """Assemble kt2 / v65 from the 8 per-core kvT outputs (layout only)."""
    kvs = [np.asarray(results[c]["kvT"]) for c in CORES]
    kT_full = np.concatenate([kv[0:64] for kv in kvs], axis=1)   # [64, B*S]
    kt2 = np.ascontiguousarray(
        np.concatenate([kT_full[:, :2 * S], kT_full[:, 2 * S:]], axis=0))
    v_full = np.concatenate([kv[64:128] for kv in kvs], axis=1).T  # [B*S, 64]
    v65 = np.empty((B * S, DQ + 1), dtype=FP16)
    v65[:, :DQ] = v_full
    v65[:, DQ] = np.asarray(1.0, dtype=FP16)
    v65 = np.ascontiguousarray(
        v65.reshape(GK, 128, DQ + 1).transpose(1, 0, 2).reshape(128, -1))
    return kt2, v65


def make_in_maps_l2(kt2, v65, tgt, mask, wq, bq, bv):
    tgt = np.asarray(tgt)
    mask = np.ascontiguousarray(mask, dtype=F32)
    wq16 = np.asarray(wq).astype(FP16)
    bq = np.ascontiguousarray(bq, dtype=F32)
    maps = []
    for c in CORES:
        m = {
            "kt2": kt2, "v65": v65,
            "tgtT": np.ascontiguousarray(
                tgt[:, c * TS:(c + 1) * TS, :].transpose(0, 2, 1)
            ).astype(FP16),
            "wq": wq16, "bq": bq,
        }
        masknT = mask[c * TS:(c + 1) * TS, :].T  # [S, TS]: [s, t]
        if N_ASG:
            m["maskA"] = np.ascontiguousarray(np.concatenate(
                [masknT[g * 128:(g + 1) * 128] for g in ACT_SGS],
                axis=0)).astype(FP16)
        if N_DSG:
            bm = np.concatenate(
                [masknT[g * 128:(g + 1) * 128] for g in DVE_SGS], axis=0)
            m["bmask"] = np.ascontiguousarray(
                np.rint(bm * A16 + B16C).astype(np.int16))
        maps.append(m)
    return maps


def kernel(src, tgt, mask, wq, bq, wk, bk, wv, bv):
    from concourse.bass_utils import run_bass_kernel_spmd

    res1 = run_bass_kernel_spmd(_get_l1(), make_in_maps_l1(src, wk, wv),
                                core_ids=CORES)
    kt2, v65 = glue_l1_outputs(res1.results)
    res2 = run_bass_kernel_spmd(
        _get_l2(), make_in_maps_l2(kt2, v65, tgt, mask, wq, bq, bv),
        core_ids=CORES)
    bv = np.ascontiguousarray(bv, dtype=F32)
    out = np.empty((B, S, DQ), dtype=F32)
    for c in CORES:
        oc = np.asarray(res2.results[c]["o"])          # [B, 65, TS] f32
        att = oc[:, :DQ, :] / oc[:, DQ:DQ + 1, :]      # softmax division
        out[:, c * TS:(c + 1) * TS, :] = \
            att.transpose(0, 2, 1) + bv[None, None, :]
    return out


# revision 3
# speedup vs baseline: 1.4111x; 1.1164x over previous
"""Trainium2 Bass kernel for single-head cross-attention with additive mask.

Computation (matches the reference):
    q = tgt @ wq + bq
    k = src @ wk (+ bk dropped: softmax cancels a per-row constant exactly)
    v = src @ wv (bv applied on host in the epilogue)
    s = (q k^T + mask) / sqrt(DQ)
    out = softmax(s) @ v + bv

Two SPMD launches on 8 cores (all matmul inputs fp16, fp32 PSUM accum):
  L1: each core projects K and V for 1/8 of the global (B*S) src rows in a
      single fused matmul (wk|wv concatenated -> kvT [128, 2048] fp16 out).
  host: pure layout glue -- assembles kt2 (d-major K) and v65 (V with an
      appended ones column for the softmax denominator).
  L2: tgt sharded 8 ways; core c handles tgt rows [c*512,(c+1)*512) of every
      batch so its mask slice is read from HBM exactly once.

L2 computes scores transposed (src-block on PSUM partitions) batch-column
outer: column c processes batches {c, c+2} whose QK matmuls use disjoint
64-row PE tiles (tile_position) and run concurrently.  The additive mask
enters PSUM ahead of QK via an identity-weight matmul (start=True), so the
scalar engine reads (qk+mask) straight from PSUM and emits fp16 exp at
scale=1/8.  A subset of src-blocks (DVE_SGS) instead computes exp on the
otherwise-idle vector engine with a Schraudolph bit-trick in the fp16 bit
domain: bits16 = int16(qk*A16 + bmask), where bmask (host-baked int16)
carries mask*A16 + (15-sigma)*1024; the int16 tile bitcast to fp16 IS the
approximate exp (rel err ~3%, diluted to ~1.2e-2 end-to-end).  PV matmul
emission trails QK by PIPE_LAG blocks so the tensor engine never idles
waiting for an exp (keeps HAM un-throttled).  All DMA rides the two HW-DGE
engines (sync/scalar), issued in need-order so the q projection's tgt
slices land first.  PV accumulates fp32 in PSUM with a 65th "ones" output
row; the final division by the softmax denominator (+bv) runs on the host.
"""
import numpy as np

B, S, D, DQ = 4, 4096, 1024, 64
NCORES = 8
TS = S // NCORES            # 512 tgt rows per core
SR = (B * S) // NCORES      # 2048 global src rows per core (L1)
SB = S // 128               # 32 src blocks per batch
GK = B * SB                 # 128 global src blocks
CORES = list(range(NCORES))
F32 = np.float32
FP16 = np.float16
PIPE_LAG = 2

# --- DVE fast-exp (Schraudolph in fp16 bit domain) ---
N_DVE_SG = 14               # how many of the 32 src-blocks use the DVE path
SIGMA = 0.0579
A16 = (2.0 ** 10) * np.log2(np.e) / 8.0
B16C = (2.0 ** 10) * (15.0 - SIGMA)
# spread the DVE blocks evenly through the sg loop so ACT/DVE interleave
DVE_SGS = tuple(g for g in range(SB)
                if (g + 1) * N_DVE_SG // SB > g * N_DVE_SG // SB)
ACT_SGS = tuple(g for g in range(SB) if g not in DVE_SGS)
N_ASG, N_DSG = len(ACT_SGS), len(DVE_SGS)
# position of each sg within its path's packed mask array
_MASK_IDX = {g: i for i, g in enumerate(ACT_SGS)}
_MASK_IDX.update({g: i for i, g in enumerate(DVE_SGS)})

_CACHE = {}


def _build_l1():
    import concourse.mybir as mybir
    import concourse.tile as tile
    from concourse import bacc

    f32 = mybir.dt.float32
    fp16 = mybir.dt.float16

    nc = bacc.Bacc("TRN2", target_bir_lowering=False, debug=False,
                   num_devices=NCORES)
    srcT = nc.dram_tensor("srcT", [D, SR], fp16, kind="ExternalInput")
    wkv = nc.dram_tensor("wkv", [D, 128], fp16, kind="ExternalInput")
    kvT = nc.dram_tensor("kvT", [128, SR], fp16, kind="ExternalOutput")

    with tile.TileContext(nc) as tc:
        with (
            tc.tile_pool(name="const", bufs=1) as constp,
            tc.tile_pool(name="big", bufs=1) as bigp,
            tc.tile_pool(name="stream", bufs=2) as streamp,
            tc.tile_pool(name="pp", bufs=1, space="PSUM") as pp,
        ):
            wkv_sb = constp.tile([128, 8 * 128], fp16)
            nc.sync.dma_start(
                out=wkv_sb.rearrange("p (j m) -> p j m", m=128),
                in_=wkv.rearrange("(j p) m -> p j m", p=128))
            sts = []
            for j in range(8):
                st = streamp.tile([128, SR], fp16, tag="xs", bufs=8,
                                  name=f"st{j}")
                nc.sync.dma_start(out=st[:], in_=srcT[j * 128:(j + 1) * 128, :])
                sts.append(st)
            kv_ps = pp.tile([128, SR], f32)
            for j in range(8):
                for q in range(4):
                    nc.tensor.matmul(
                        kv_ps[:, q * 512:(q + 1) * 512],
                        lhsT=wkv_sb[:, j * 128:(j + 1) * 128],
                        rhs=sts[j][:, q * 512:(q + 1) * 512],
                        start=(j == 0), stop=(j == 7))
            kv_sb = bigp.tile([128, SR], fp16)
            for q in range(4):
                eng = nc.scalar if q % 2 == 0 else nc.vector
                if q % 2 == 0:
                    nc.scalar.copy(kv_sb[:, q * 512:(q + 1) * 512],
                                   kv_ps[:, q * 512:(q + 1) * 512])
                else:
                    nc.vector.tensor_copy(kv_sb[:, q * 512:(q + 1) * 512],
                                          kv_ps[:, q * 512:(q + 1) * 512])
                nc.sync.dma_start(out=kvT[:, q * 512:(q + 1) * 512],
                                  in_=kv_sb[:, q * 512:(q + 1) * 512])
    nc.compile()
    return nc


def _build_l2():
    import concourse.mybir as mybir
    import concourse.tile as tile
    from concourse import bacc
    from concourse.masks import make_identity

    f32 = mybir.dt.float32
    fp16 = mybir.dt.float16
    i16 = mybir.dt.int16
    AF = mybir.ActivationFunctionType
    ALU = mybir.AluOpType

    nc = bacc.Bacc("TRN2", target_bir_lowering=False, debug=False,
                   num_devices=NCORES)
    # kt2: partitions 0-63 = kT of batches 0|1; 64-127 = batches 2|3
    kt2d = nc.dram_tensor("kt2", [128, 2 * S], fp16, kind="ExternalInput")
    # v65 layout: row p, cols (kg, c): element = v[kg*128 + p, c] | ones
    v65d = nc.dram_tensor("v65", [128, GK * (DQ + 1)], fp16,
                          kind="ExternalInput")
    tgtT = nc.dram_tensor("tgtT", [B, D, TS], fp16, kind="ExternalInput")
    # packed transposed mask rows for the ACT-path src blocks (fp16)
    if N_ASG:
        maskA = nc.dram_tensor("maskA", [N_ASG * 128, TS], fp16,
                               kind="ExternalInput")
    # packed Schraudolph bias rows for the DVE-path src blocks (int16)
    if N_DSG:
        bmaskd = nc.dram_tensor("bmask", [N_DSG * 128, TS], i16,
                                kind="ExternalInput")
    wq = nc.dram_tensor("wq", [D, DQ], fp16, kind="ExternalInput")
    bq = nc.dram_tensor("bq", [DQ], f32, kind="ExternalInput")
    # out rows 0-63: (attn @ v)^T numerator; row 64: softmax denominator
    o = nc.dram_tensor("o", [B, DQ + 1, TS], f32, kind="ExternalOutput")

    with tile.TileContext(nc) as tc:
        with (
            tc.tile_pool(name="const", bufs=1) as constp,
            tc.tile_pool(name="big", bufs=1) as bigp,
            tc.tile_pool(name="stream", bufs=2) as streamp,
            tc.tile_pool(name="pp", bufs=1, space="PSUM") as pp,
        ):
            # ---- constants + all input DMA, issued in need-order on sync
            wq_sb = constp.tile([128, 8 * DQ], fp16)
            nc.sync.dma_start(
                out=wq_sb.rearrange("p (j m) -> p j m", m=DQ),
                in_=wq.rearrange("(j p) m -> p j m", p=128))
            bq_sb = constp.tile([128, 1], f32)
            nc.sync.dma_start(out=bq_sb[0:64, :],
                              in_=bq.rearrange("(p o) -> p o", o=1))
            nc.sync.dma_start(out=bq_sb[64:128, :],
                              in_=bq.rearrange("(p o) -> p o", o=1))
            ident = constp.tile([128, 128], fp16)
            make_identity(nc, ident[:])

            kT2 = bigp.tile([128, 2 * S], fp16)
            v2 = bigp.tile([128, GK * (DQ + 1)], fp16)
            VQ = 32 * (DQ + 1)
            if N_ASG:
                maskA_sb = bigp.tile([128, N_ASG * TS], fp16)
                mav = maskA_sb.rearrange("p (g t) -> p g t", t=TS)
            if N_DSG:
                bmask_sb = bigp.tile([128, N_DSG * TS], i16)
                bmv = bmask_sb.rearrange("p (g t) -> p g t", t=TS)
            tgs = {}
            for b in (0, 2, 1, 3):
                for half in range(2):
                    tgs[b, half] = streamp.tile(
                        [128, SR], fp16, tag="tg", bufs=8,
                        name=f"tg{b}_{half}")

            def load_tg(b):
                for half in range(2):
                    nc.sync.dma_start(
                        out=tgs[b, half].rearrange("p (j t) -> p j t", t=TS),
                        in_=tgtT[b, half * 512:(half + 1) * 512, :]
                        .rearrange("(j p) t -> p j t", p=128))

            def load_kt2(lo, hi):
                nc.sync.dma_start(out=kT2[:, lo:hi], in_=kt2d[:, lo:hi])

            def load_v2(bb):
                nc.sync.dma_start(out=v2[:, bb * VQ:(bb + 1) * VQ],
                                  in_=v65d[:, bb * VQ:(bb + 1) * VQ])

            def load_maskA(lo, hi):
                hi = min(hi, N_ASG)
                if N_ASG and lo < hi:
                    nc.sync.dma_start(
                        out=mav[:, lo:hi, :],
                        in_=maskA[lo * 128:hi * 128, :]
                        .rearrange("(g p) t -> p g t", p=128))

            def load_bmask(lo, hi):
                hi = min(hi, N_DSG)
                if N_DSG and lo < hi:
                    nc.sync.dma_start(
                        out=bmv[:, lo:hi, :],
                        in_=bmaskd[lo * 128:hi * 128, :]
                        .rearrange("(g p) t -> p g t", p=128))

            load_tg(0)
            load_tg(2)
            load_kt2(0, 2048)               # column 0, sg 0-15
            load_maskA(0, 4)
            load_bmask(0, 4)
            load_v2(0)
            load_v2(2)
            load_maskA(4, 12)
            load_bmask(4, 12)
            load_kt2(2048, 4096)            # column 0, sg 16-31
            load_tg(1)
            load_tg(3)
            load_maskA(12, N_ASG)
            load_bmask(12, N_DSG)
            load_kt2(4096, 6144)            # column 1
            load_kt2(6144, 8192)
            load_v2(1)
            load_v2(3)

            # ---- q projection (column order; b1/b3 emitted mid-loop below)
            qT_sb = bigp.tile([128, 2 * TS], fp16)

            def qproj(b):
                pb, colb = (b // 2) * 64, (b % 2) * TS
                q_ps = pp.tile([128, TS], f32, tag="qk", bufs=3,
                               name=f"q_ps{b}")
                for j in range(8):
                    nc.tensor.matmul(
                        q_ps[pb:pb + 64, :],
                        lhsT=wq_sb[:, j * DQ:(j + 1) * DQ],
                        rhs=tgs[b, j // 4][:, (j % 4) * TS:(j % 4 + 1) * TS],
                        start=(j == 0), stop=(j == 7),
                        tile_position=(0, pb))
                nc.scalar.activation(
                    qT_sb[pb:pb + 64, colb:colb + TS], q_ps[pb:pb + 64, :],
                    AF.Identity, bias=bq_sb[pb:pb + 64, :])

            qproj(0)
            qproj(2)

            # ---- attention main loop: batch-column outer; column c handles
            # batches {c, c+2} on disjoint 64-row PE tiles.  PV emission
            # trails by PIPE_LAG so the PE never waits on an exp.
            for col in range(2):
                pv = [pp.tile([DQ + 1, TS], f32, tag=f"pv{h}",
                              name=f"pv{col}_{h}") for h in range(2)]
                pts = {}

                def emit_qk_exp(sg, col=col, pts=pts):
                    dve = sg in DVE_SGS
                    mi = _MASK_IDX[sg]
                    qkt = pp.tile([128, 2 * TS], f32, tag="qk", bufs=3,
                                  name=f"qkt{col}_{sg}")
                    if not dve:
                        for half in range(2):
                            nc.tensor.matmul(
                                qkt[:, half * TS:(half + 1) * TS],
                                lhsT=ident[:],
                                rhs=maskA_sb[:, mi * TS:(mi + 1) * TS],
                                start=True, stop=False)
                    for half in range(2):
                        nc.tensor.matmul(
                            qkt[:, half * TS:(half + 1) * TS],
                            lhsT=kT2[half * 64:half * 64 + 64,
                                     col * S + sg * 128:
                                     col * S + sg * 128 + 128],
                            rhs=qT_sb[half * 64:half * 64 + 64,
                                      col * TS:(col + 1) * TS],
                            start=dve, stop=True,
                            tile_position=(half * 64, 0))
                    pt = streamp.tile([128, 2 * TS], fp16, tag="P", bufs=6,
                                      name=f"pt{col}_{sg}")
                    if not dve:
                        nc.scalar.activation(pt[:], qkt[:], AF.Exp,
                                             scale=0.125)
                    else:
                        for half in range(2):
                            nc.vector.scalar_tensor_tensor(
                                out=pt[:, half * TS:(half + 1) * TS]
                                .bitcast(i16),
                                in0=qkt[:, half * TS:(half + 1) * TS],
                                scalar=float(A16),
                                in1=bmask_sb[:, mi * TS:(mi + 1) * TS],
                                op0=ALU.mult, op1=ALU.add)
                    pts[sg] = pt

                def emit_pv(sg, col=col, pv=pv, pts=pts):
                    pt = pts.pop(sg)
                    for half in range(2):
                        kg = (col + 2 * half) * SB + sg
                        nc.tensor.matmul(
                            pv[half][:],
                            lhsT=v2[:, kg * (DQ + 1):(kg + 1) * (DQ + 1)],
                            rhs=pt[:, half * TS:(half + 1) * TS],
                            start=(sg == 0), stop=(sg == SB - 1))

                for sg in range(SB):
                    emit_qk_exp(sg)
                    if sg >= PIPE_LAG:
                        emit_pv(sg - PIPE_LAG)
                    if col == 0 and sg == 8:
                        qproj(1)
                    if col == 0 and sg == 10:
                        qproj(3)
                for sg in range(SB - PIPE_LAG, SB):
                    emit_pv(sg)

                for half in range(2):
                    ob = streamp.tile([DQ + 1, TS], f32, tag="ob", bufs=4,
                                      name=f"ob{col}_{half}")
                    if half == 0:
                        nc.scalar.copy(ob[:], pv[half][:])
                    else:
                        nc.vector.tensor_copy(ob[:], pv[half][:])
                    nc.scalar.dma_start(out=o[col + 2 * half], in_=ob[:])
    nc.compile()
    return nc


def _get_l1():
    if "l1" not in _CACHE:
        _CACHE["l1"] = _build_l1()
    return _CACHE["l1"]


def _get_l2():
    if "l2" not in _CACHE:
        _CACHE["l2"] = _build_l2()
    return _CACHE["l2"]


def make_in_maps_l1(src, wk, wv):
    src16 = np.asarray(src).astype(FP16).reshape(B * S, D)
    wkv = np.concatenate([np.asarray(wk), np.asarray(wv)],
                         axis=1).astype(FP16)
    return [{
        "srcT": np.ascontiguousarray(src16[c * SR:(c + 1) * SR, :].T),
        "wkv": wkv,
    } for c in CORES]


def glue_l1_outputs(results):
    """Assemble kt2 / v65 from the 8 per-core kvT outputs (layout only)."""
    kvs = [np.asarray(results[c]["kvT"]) for c in CORES]
    kT_full = np.concatenate([kv[0:64] for kv in kvs], axis=1)   # [64, B*S]
    kt2 = np.ascontiguousarray(
        np.concatenate([kT_full[:, :2 * S], kT_full[:, 2 * S:]], axis=0))
    v_full = np.concatenate([kv[64:128] for kv in kvs], axis=1).T  # [B*S, 64]
    v65 = np.empty((B * S, DQ + 1), dtype=FP16)
    v65[:, :DQ] = v_full
    v65[:, DQ] = np.asarray(1.0, dtype=FP16)
    v65 = np.ascontiguousarray(
        v65.reshape(GK, 128, DQ + 1).transpose(1, 0, 2).reshape(128, -1))
    return kt2, v65


def make_in_maps_l2(kt2, v65, tgt, mask, wq, bq, bv):
    tgt = np.asarray(tgt)
    mask = np.ascontiguousarray(mask, dtype=F32)
    wq16 = np.asarray(wq).astype(FP16)
    bq = np.ascontiguousarray(bq, dtype=F32)
    maps = []
    for c in CORES:
        m = {
            "kt2": kt2, "v65": v65,
            "tgtT": np.ascontiguousarray(
                tgt[:, c * TS:(c + 1) * TS, :].transpose(0, 2, 1)
            ).astype(FP16),
            "wq": wq16, "bq": bq,
        }
        masknT = mask[c * TS:(c + 1) * TS, :].T  # [S, TS]: [s, t]
        if N_ASG:
            m["maskA"] = np.ascontiguousarray(np.concatenate(
                [masknT[g * 128:(g + 1) * 128] for g in ACT_SGS],
                axis=0)).astype(FP16)
        if N_DSG:
            bm = np.concatenate(
                [masknT[g * 128:(g + 1) * 128] for g in DVE_SGS], axis=0)
            m["bmask"] = np.ascontiguousarray(
                np.rint(bm * A16 + B16C).astype(np.int16))
        maps.append(m)
    return maps


def kernel(src, tgt, mask, wq, bq, wk, bk, wv, bv):
    from concourse.bass_utils import run_bass_kernel_spmd

    res1 = run_bass_kernel_spmd(_get_l1(), make_in_maps_l1(src, wk, wv),
                                core_ids=CORES)
    kt2, v65 = glue_l1_outputs(res1.results)
    res2 = run_bass_kernel_spmd(
        _get_l2(), make_in_maps_l2(kt2, v65, tgt, mask, wq, bq, bv),
        core_ids=CORES)
    bv = np.ascontiguousarray(bv, dtype=F32)
    out = np.empty((B, S, DQ), dtype=F32)
    for c in CORES:
        oc = np.asarray(res2.results[c]["o"])          # [B, 65, TS] f32
        att = oc[:, :DQ, :] / oc[:, DQ:DQ + 1, :]      # softmax division
        out[:, c * TS:(c + 1) * TS, :] = \
            att.transpose(0, 2, 1) + bv[None, None, :]
    return out


# revision 12
# speedup vs baseline: 1.6119x; 1.1423x over previous
"""Trainium2 Bass kernel for single-head cross-attention with additive mask.

Computation (matches the reference):
    q = tgt @ wq + bq
    k = src @ wk (+ bk dropped: softmax cancels a per-row constant exactly)
    v = src @ wv (bv applied on host in the epilogue)
    s = (q k^T + mask) / sqrt(DQ)
    out = softmax(s) @ v + bv

Two SPMD launches on 8 cores (all matmul inputs fp16, fp32 PSUM accum):
  L1: each core projects K and V for 1/8 of the global (B*S) src rows in a
      single fused matmul (wk|wv concatenated -> kvT [128, 2048] fp16 out).
  host: pure layout glue -- assembles kt2 (d-major K) and v65 (V with an
      appended ones column for the softmax denominator).
  L2: tgt sharded 8 ways; core c handles tgt rows [c*512,(c+1)*512) of every
      batch so its mask slice is read from HBM exactly once.

L2 computes scores transposed (src-block on PSUM partitions) batch-column
outer: column c processes batches {c, c+2} whose QK matmuls use disjoint
64-row PE tiles (tile_position) and run concurrently.  The additive mask
enters PSUM ahead of QK via an identity-weight matmul (start=True), so the
scalar engine reads (qk+mask) straight from PSUM and emits fp16 exp at
scale=1/8.  A subset of src-blocks (DVE_SGS) instead computes exp on the
otherwise-idle vector engine with a Schraudolph bit-trick in the fp16 bit
domain: bits16 = int16(qk*A16 + bmask), where bmask (host-baked int16)
carries mask*A16 + (15-sigma)*1024; the int16 tile bitcast to fp16 IS the
approximate exp (rel err ~3%, diluted to ~1.2e-2 end-to-end).  PV matmul
emission trails QK by PIPE_LAG blocks so the tensor engine never idles
waiting for an exp (keeps HAM un-throttled).  All DMA rides the two HW-DGE
engines (sync/scalar), issued in need-order so the q projection's tgt
slices land first.  PV accumulates fp32 in PSUM with a 65th "ones" output
row; the final division by the softmax denominator (+bv) runs on the host.
"""
import numpy as np

B, S, D, DQ = 4, 4096, 1024, 64
NCORES = 8
TS = S // NCORES            # 512 tgt rows per core
SR = (B * S) // NCORES      # 2048 global src rows per core (L1)
SB = S // 128               # 32 src blocks per batch
GK = B * SB                 # 128 global src blocks
CORES = list(range(NCORES))
F32 = np.float32
FP16 = np.float16
PIPE_LAG = 5

# --- DVE fast-exp (Schraudolph in fp16 bit domain) ---
N_DVE_SG = 14               # src-blocks on the DVE Schraudolph path
N_GP_SG = 8                 # src-blocks on the GPSIMD emask-multiply path
SIGMA = 0.035
A16 = (2.0 ** 10) * np.log2(np.e) / 8.0
B16C = (2.0 ** 10) * (15.0 - SIGMA)
# spread the DVE blocks evenly through the sg loop so ACT/DVE interleave
DVE_SGS = tuple(g for g in range(SB)
                if (g + 1) * N_DVE_SG // SB > g * N_DVE_SG // SB)
_REST = tuple(g for g in range(SB) if g not in DVE_SGS)
GP_SGS = tuple(_REST[i] for i in range(0, 2 * N_GP_SG, 2))  # alternate
ACT_SGS = tuple(g for g in _REST if g not in GP_SGS)
N_ASG, N_DSG, N_GSG = len(ACT_SGS), len(DVE_SGS), len(GP_SGS)
# position of each sg within its path's packed mask array
_MASK_IDX = {g: i for i, g in enumerate(ACT_SGS)}
_MASK_IDX.update({g: i for i, g in enumerate(DVE_SGS)})
_MASK_IDX.update({g: i for i, g in enumerate(GP_SGS)})

_CACHE = {}


def _build_l1():
    import concourse.mybir as mybir
    import concourse.tile as tile
    from concourse import bacc

    f32 = mybir.dt.float32
    fp16 = mybir.dt.float16

    nc = bacc.Bacc("TRN2", target_bir_lowering=False, debug=False,
                   num_devices=NCORES)
    srcT = nc.dram_tensor("srcT", [D, SR], fp16, kind="ExternalInput")
    wkv = nc.dram_tensor("wkv", [D, 128], fp16, kind="ExternalInput")
    kvT = nc.dram_tensor("kvT", [128, SR], fp16, kind="ExternalOutput")

    with tile.TileContext(nc) as tc:
        with (
            tc.tile_pool(name="const", bufs=1) as constp,
            tc.tile_pool(name="big", bufs=1) as bigp,
            tc.tile_pool(name="stream", bufs=2) as streamp,
            tc.tile_pool(name="pp", bufs=1, space="PSUM") as pp,
        ):
            wkv_sb = constp.tile([128, 8 * 128], fp16)
            nc.sync.dma_start(
                out=wkv_sb.rearrange("p (j m) -> p j m", m=128),
                in_=wkv.rearrange("(j p) m -> p j m", p=128))
            sts = []
            for j in range(8):
                st = streamp.tile([128, SR], fp16, tag="xs", bufs=8,
                                  name=f"st{j}")
                nc.sync.dma_start(out=st[:], in_=srcT[j * 128:(j + 1) * 128, :])
                sts.append(st)
            kv_ps = pp.tile([128, SR], f32)
            for j in range(8):
                for q in range(4):
                    nc.tensor.matmul(
                        kv_ps[:, q * 512:(q + 1) * 512],
                        lhsT=wkv_sb[:, j * 128:(j + 1) * 128],
                        rhs=sts[j][:, q * 512:(q + 1) * 512],
                        start=(j == 0), stop=(j == 7))
            kv_sb = bigp.tile([128, SR], fp16)
            for q in range(4):
                eng = nc.scalar if q % 2 == 0 else nc.vector
                if q % 2 == 0:
                    nc.scalar.copy(kv_sb[:, q * 512:(q + 1) * 512],
                                   kv_ps[:, q * 512:(q + 1) * 512])
                else:
                    nc.vector.tensor_copy(kv_sb[:, q * 512:(q + 1) * 512],
                                          kv_ps[:, q * 512:(q + 1) * 512])
                nc.sync.dma_start(out=kvT[:, q * 512:(q + 1) * 512],
                                  in_=kv_sb[:, q * 512:(q + 1) * 512])
    nc.compile()
    return nc


def _build_l2():
    import concourse.mybir as mybir
    import concourse.tile as tile
    from concourse import bacc
    from concourse.masks import make_identity

    f32 = mybir.dt.float32
    fp16 = mybir.dt.float16
    i16 = mybir.dt.int16
    AF = mybir.ActivationFunctionType
    ALU = mybir.AluOpType

    nc = bacc.Bacc("TRN2", target_bir_lowering=False, debug=False,
                   num_devices=NCORES)
    # kt2: partitions 0-63 = kT of batches 0|1; 64-127 = batches 2|3
    kt2d = nc.dram_tensor("kt2", [128, 2 * S], fp16, kind="ExternalInput")
    # v65 layout: row p, cols (kg, c): element = v[kg*128 + p, c] | ones
    v65d = nc.dram_tensor("v65", [128, GK * (DQ + 1)], fp16,
                          kind="ExternalInput")
    tgtT = nc.dram_tensor("tgtT", [B, D, TS], fp16, kind="ExternalInput")
    # packed transposed mask rows for the ACT-path src blocks (fp16)
    if N_ASG:
        maskA = nc.dram_tensor("maskA", [N_ASG * 128, TS], fp16,
                               kind="ExternalInput")
    # packed Schraudolph bias rows for the DVE-path src blocks (int16)
    if N_DSG:
        bmaskd = nc.dram_tensor("bmask", [N_DSG * 128, TS], i16,
                                kind="ExternalInput")
    # packed exp(mask/8) rows for the GPSIMD-multiply src blocks (fp16)
    if N_GSG:
        emaskd = nc.dram_tensor("emask", [N_GSG * 128, TS], fp16,
                                kind="ExternalInput")
    wq = nc.dram_tensor("wq", [D, DQ], fp16, kind="ExternalInput")
    bq = nc.dram_tensor("bq", [DQ], f32, kind="ExternalInput")
    # out rows 0-63: (attn @ v)^T numerator; row 64: softmax denominator
    o = nc.dram_tensor("o", [B, DQ + 1, TS], f32, kind="ExternalOutput")

    with tile.TileContext(nc) as tc:
        with (
            tc.tile_pool(name="const", bufs=1) as constp,
            tc.tile_pool(name="big", bufs=1) as bigp,
            tc.tile_pool(name="stream", bufs=2) as streamp,
            tc.tile_pool(name="pp", bufs=1, space="PSUM") as pp,
        ):
            # ---- constants + all input DMA, issued in need-order on sync
            wq_sb = constp.tile([128, 8 * DQ], fp16)
            nc.sync.dma_start(
                out=wq_sb.rearrange("p (j m) -> p j m", m=DQ),
                in_=wq.rearrange("(j p) m -> p j m", p=128))
            bq_sb = constp.tile([128, 1], f32)
            nc.sync.dma_start(out=bq_sb[0:64, :],
                              in_=bq.rearrange("(p o) -> p o", o=1))
            nc.sync.dma_start(out=bq_sb[64:128, :],
                              in_=bq.rearrange("(p o) -> p o", o=1))
            ident = constp.tile([128, 128], fp16)
            make_identity(nc, ident[:])

            kT2 = bigp.tile([128, 2 * S], fp16)
            v2 = bigp.tile([128, GK * (DQ + 1)], fp16)
            VQ = 32 * (DQ + 1)
            if N_ASG:
                maskA_sb = bigp.tile([128, N_ASG * TS], fp16)
                mav = maskA_sb.rearrange("p (g t) -> p g t", t=TS)
            if N_DSG:
                bmask_sb = bigp.tile([128, N_DSG * TS], i16)
                bmv = bmask_sb.rearrange("p (g t) -> p g t", t=TS)
            if N_GSG:
                emask_sb = bigp.tile([128, N_GSG * TS], fp16)
                emv = emask_sb.rearrange("p (g t) -> p g t", t=TS)
            tgs = {}
            for b in (0, 2, 1, 3):
                for half in range(2):
                    tgs[b, half] = streamp.tile(
                        [128, SR], fp16, tag="tg", bufs=8,
                        name=f"tg{b}_{half}")

            def load_tg(b):
                for half in range(2):
                    nc.sync.dma_start(
                        out=tgs[b, half].rearrange("p (j t) -> p j t", t=TS),
                        in_=tgtT[b, half * 512:(half + 1) * 512, :]
                        .rearrange("(j p) t -> p j t", p=128))

            def load_kt2(lo, hi):
                nc.sync.dma_start(out=kT2[:, lo:hi], in_=kt2d[:, lo:hi])

            def load_v2(bb):
                nc.sync.dma_start(out=v2[:, bb * VQ:(bb + 1) * VQ],
                                  in_=v65d[:, bb * VQ:(bb + 1) * VQ])

            def load_maskA(lo, hi):
                hi = min(hi, N_ASG)
                if N_ASG and lo < hi:
                    nc.sync.dma_start(
                        out=mav[:, lo:hi, :],
                        in_=maskA[lo * 128:hi * 128, :]
                        .rearrange("(g p) t -> p g t", p=128))

            def load_bmask(lo, hi):
                hi = min(hi, N_DSG)
                if N_DSG and lo < hi:
                    nc.sync.dma_start(
                        out=bmv[:, lo:hi, :],
                        in_=bmaskd[lo * 128:hi * 128, :]
                        .rearrange("(g p) t -> p g t", p=128))

            def load_emask(lo, hi):
                hi = min(hi, N_GSG)
                if N_GSG and lo < hi:
                    nc.sync.dma_start(
                        out=emv[:, lo:hi, :],
                        in_=emaskd[lo * 128:hi * 128, :]
                        .rearrange("(g p) t -> p g t", p=128))

            load_tg(0)
            load_tg(2)
            load_kt2(0, 2048)               # column 0, sg 0-15
            load_maskA(0, 4)
            load_bmask(0, 4)
            load_emask(0, 4)
            load_v2(0)
            load_v2(2)
            load_maskA(4, 12)
            load_bmask(4, 12)
            load_emask(4, N_GSG)
            load_kt2(2048, 4096)            # column 0, sg 16-31
            load_tg(1)
            load_tg(3)
            load_maskA(12, N_ASG)
            load_bmask(12, N_DSG)
            load_kt2(4096, 6144)            # column 1
            load_kt2(6144, 8192)
            load_v2(1)
            load_v2(3)

            # ---- q projection (column order; b1/b3 emitted mid-loop below)
            qT_sb = bigp.tile([128, 2 * TS], fp16)

            def qproj(b):
                pb, colb = (b // 2) * 64, (b % 2) * TS
                q_ps = pp.tile([128, TS], f32, tag="qk", bufs=3,
                               name=f"q_ps{b}")
                for j in range(8):
                    nc.tensor.matmul(
                        q_ps[pb:pb + 64, :],
                        lhsT=wq_sb[:, j * DQ:(j + 1) * DQ],
                        rhs=tgs[b, j // 4][:, (j % 4) * TS:(j % 4 + 1) * TS],
                        start=(j == 0), stop=(j == 7))
                nc.scalar.activation(
                    qT_sb[pb:pb + 64, colb:colb + TS], q_ps[pb:pb + 64, :],
                    AF.Identity, bias=bq_sb[pb:pb + 64, :])

            qproj(0)
            qproj(2)

            # ---- attention main loop: batch-column outer; column c handles
            # batches {c, c+2} on disjoint 64-row PE tiles.  PV emission
            # trails by PIPE_LAG so the PE never waits on an exp.
            for col in range(2):
                pv = [pp.tile([DQ + 1, TS], f32, tag=f"pv{h}",
                              name=f"pv{col}_{h}") for h in range(2)]
                pts = {}

                def emit_qk_exp(sg, col=col, pts=pts):
                    dve = sg in DVE_SGS
                    gp = sg in GP_SGS
                    mi = _MASK_IDX[sg]
                    qkt = pp.tile([128, 2 * TS], f32, tag="qk", bufs=3,
                                  name=f"qkt{col}_{sg}")
                    if not (dve or gp):
                        for half in range(2):
                            nc.tensor.matmul(
                                qkt[:, half * TS:(half + 1) * TS],
                                lhsT=ident[:],
                                rhs=maskA_sb[:, mi * TS:(mi + 1) * TS],
                                start=True, stop=False)
                    for half in range(2):
                        nc.tensor.matmul(
                            qkt[:, half * TS:(half + 1) * TS],
                            lhsT=kT2[half * 64:half * 64 + 64,
                                     col * S + sg * 128:
                                     col * S + sg * 128 + 128],
                            rhs=qT_sb[half * 64:half * 64 + 64,
                                      col * TS:(col + 1) * TS],
                            start=(dve or gp), stop=True)
                    pt = streamp.tile([128, 2 * TS], fp16, tag="P", bufs=8,
                                      name=f"pt{col}_{sg}")
                    if dve:
                        for half in range(2):
                            nc.vector.scalar_tensor_tensor(
                                out=pt[:, half * TS:(half + 1) * TS]
                                .bitcast(i16),
                                in0=qkt[:, half * TS:(half + 1) * TS],
                                scalar=float(A16),
                                in1=bmask_sb[:, mi * TS:(mi + 1) * TS],
                                op0=ALU.mult, op1=ALU.add)
                    elif gp:
                        es = streamp.tile([128, 2 * TS], fp16, tag="E",
                                          bufs=4, name=f"es{col}_{sg}")
                        nc.scalar.activation(es[:], qkt[:], AF.Exp,
                                             scale=0.125)
                        for half in range(2):
                            nc.gpsimd.tensor_tensor(
                                out=pt[:, half * TS:(half + 1) * TS],
                                in0=es[:, half * TS:(half + 1) * TS],
                                in1=emask_sb[:, mi * TS:(mi + 1) * TS],
                                op=ALU.mult)
                    else:
                        nc.scalar.activation(pt[:], qkt[:], AF.Exp,
                                             scale=0.125)
                    pts[sg] = pt

                def emit_pv(sg, col=col, pv=pv, pts=pts):
                    pt = pts.pop(sg)
                    for half in range(2):
                        kg = (col + 2 * half) * SB + sg
                        nc.tensor.matmul(
                            pv[half][:],
                            lhsT=v2[:, kg * (DQ + 1):(kg + 1) * (DQ + 1)],
                            rhs=pt[:, half * TS:(half + 1) * TS],
                            start=(sg == 0), stop=(sg == SB - 1))

                for sg in range(SB):
                    emit_qk_exp(sg)
                    if sg >= PIPE_LAG:
                        emit_pv(sg - PIPE_LAG)
                    if col == 0 and sg == 8:
                        qproj(1)
                    if col == 0 and sg == 10:
                        qproj(3)
                for sg in range(SB - PIPE_LAG, SB):
                    emit_pv(sg)

                for half in range(2):
                    ob = streamp.tile([DQ + 1, TS], f32, tag="ob", bufs=4,
                                      name=f"ob{col}_{half}")
                    if half == 0:
                        nc.scalar.copy(ob[:], pv[half][:])
                    else:
                        nc.vector.tensor_copy(ob[:], pv[half][:])
                    nc.sync.dma_start(out=o[col + 2 * half], in_=ob[:])
    nc.compile()
    return nc


def _get_l1():
    if "l1" not in _CACHE:
        _CACHE["l1"] = _build_l1()
    return _CACHE["l1"]


def _get_l2():
    if "l2" not in _CACHE:
        _CACHE["l2"] = _build_l2()
    return _CACHE["l2"]


def make_in_maps_l1(src, wk, wv):
    src16 = np.asarray(src).astype(FP16).reshape(B * S, D)
    wkv = np.concatenate([np.asarray(wk), np.asarray(wv)],
                         axis=1).astype(FP16)
    return [{
        "srcT": np.ascontiguousarray(src16[c * SR:(c + 1) * SR, :].T),
        "wkv": wkv,
    } for c in CORES]


def glue_l1_outputs(results):
    """Assemble kt2 / v65 from the 8 per-core kvT outputs (layout only)."""
    kvs = [np.asarray(results[c]["kvT"]) for c in CORES]
    kT_full = np.concatenate([kv[0:64] for kv in kvs], axis=1)   # [64, B*S]
    kt2 = np.ascontiguousarray(
        np.concatenate([kT_full[:, :2 * S], kT_full[:, 2 * S:]], axis=0))
    v_full = np.concatenate([kv[64:128] for kv in kvs], axis=1).T  # [B*S, 64]
    v65 = np.empty((B * S, DQ + 1), dtype=FP16)
    v65[:, :DQ] = v_full
    v65[:, DQ] = np.asarray(1.0, dtype=FP16)
    v65 = np.ascontiguousarray(
        v65.reshape(GK, 128, DQ + 1).transpose(1, 0, 2).reshape(128, -1))
    return kt2, v65


def make_in_maps_l2(kt2, v65, tgt, mask, wq, bq, bv):
    tgt = np.asarray(tgt)
    mask = np.ascontiguousarray(mask, dtype=F32)
    wq16 = np.asarray(wq).astype(FP16)
    bq = np.ascontiguousarray(bq, dtype=F32)
    maps = []
    for c in CORES:
        m = {
            "kt2": kt2, "v65": v65,
            "tgtT": np.ascontiguousarray(
                tgt[:, c * TS:(c + 1) * TS, :].transpose(0, 2, 1)
            ).astype(FP16),
            "wq": wq16, "bq": bq,
        }
        masknT = mask[c * TS:(c + 1) * TS, :].T  # [S, TS]: [s, t]
        if N_ASG:
            m["maskA"] = np.ascontiguousarray(np.concatenate(
                [masknT[g * 128:(g + 1) * 128] for g in ACT_SGS],
                axis=0)).astype(FP16)
        if N_DSG:
            bm = np.concatenate(
                [masknT[g * 128:(g + 1) * 128] for g in DVE_SGS], axis=0)
            m["bmask"] = np.ascontiguousarray(
                np.rint(bm * A16 + B16C).astype(np.int16))
        if N_GSG:
            em = np.concatenate(
                [masknT[g * 128:(g + 1) * 128] for g in GP_SGS], axis=0)
            m["emask"] = np.ascontiguousarray(
                np.exp(em * 0.125)).astype(FP16)
        maps.append(m)
    return maps


def kernel(src, tgt, mask, wq, bq, wk, bk, wv, bv):
    from concourse.bass_utils import run_bass_kernel_spmd

    res1 = run_bass_kernel_spmd(_get_l1(), make_in_maps_l1(src, wk, wv),
                                core_ids=CORES)
    kt2, v65 = glue_l1_outputs(res1.results)
    res2 = run_bass_kernel_spmd(
        _get_l2(), make_in_maps_l2(kt2, v65, tgt, mask, wq, bq, bv),
        core_ids=CORES)
    bv = np.ascontiguousarray(bv, dtype=F32)
    out = np.empty((B, S, DQ), dtype=F32)
    for c in CORES:
        oc = np.asarray(res2.results[c]["o"])          # [B, 65, TS] f32
        att = oc[:, :DQ, :] / oc[:, DQ:DQ + 1, :]      # softmax division
        out[:, c * TS:(c + 1) * TS, :] = \
            att.transpose(0, 2, 1) + bv[None, None, :]
    return out


# revision 21
# speedup vs baseline: 1.6134x; 1.0010x over previous
"""Trainium2 Bass kernel for single-head cross-attention with additive mask.

Computation (matches the reference):
    q = tgt @ wq + bq
    k = src @ wk (+ bk dropped: softmax cancels a per-row constant exactly)
    v = src @ wv (bv applied on host in the epilogue)
    s = (q k^T + mask) / sqrt(DQ)
    out = softmax(s) @ v + bv

Two SPMD launches on 8 cores (all matmul inputs fp16, fp32 PSUM accum):
  L1: each core projects K and V for 1/8 of the global (B*S) src rows in a
      single fused matmul (wk|wv concatenated -> kvT [128, 2048] fp16 out).
  host: pure layout glue -- assembles kt2 (d-major K) and v65 (V with an
      appended ones column for the softmax denominator).
  L2: tgt sharded 8 ways; core c handles tgt rows [c*512,(c+1)*512) of every
      batch so its mask slice is read from HBM exactly once.

L2 computes scores transposed (src-block on PSUM partitions) batch-column
outer: column c processes batches {c, c+2} whose QK matmuls use disjoint
64-row PE tiles (tile_position) and run concurrently.  The additive mask
enters PSUM ahead of QK via an identity-weight matmul (start=True), so the
scalar engine reads (qk+mask) straight from PSUM and emits fp16 exp at
scale=1/8.  A subset of src-blocks (DVE_SGS) instead computes exp on the
otherwise-idle vector engine with a Schraudolph bit-trick in the fp16 bit
domain: bits16 = int16(qk*A16 + bmask), where bmask (host-baked int16)
carries mask*A16 + (15-sigma)*1024; the int16 tile bitcast to fp16 IS the
approximate exp (rel err ~3%, diluted to ~1.2e-2 end-to-end).  PV matmul
emission trails QK by PIPE_LAG blocks so the tensor engine never idles
waiting for an exp (keeps HAM un-throttled).  All DMA rides the two HW-DGE
engines (sync/scalar), issued in need-order so the q projection's tgt
slices land first.  PV accumulates fp32 in PSUM with a 65th "ones" output
row; the final division by the softmax denominator (+bv) runs on the host.
"""
import numpy as np

B, S, D, DQ = 4, 4096, 1024, 64
NCORES = 8
TS = S // NCORES            # 512 tgt rows per core
SR = (B * S) // NCORES      # 2048 global src rows per core (L1)
SB = S // 128               # 32 src blocks per batch
GK = B * SB                 # 128 global src blocks
CORES = list(range(NCORES))
F32 = np.float32
FP16 = np.float16
PIPE_LAG = 5

# --- DVE fast-exp (Schraudolph in fp16 bit domain) ---
N_DVE_SG = 14               # src-blocks on the DVE Schraudolph path
N_GP_SG = 8                 # src-blocks on the GPSIMD emask-multiply path
SIGMA = 0.035
A16 = (2.0 ** 10) * np.log2(np.e) / 8.0
B16C = (2.0 ** 10) * (15.0 - SIGMA)
# spread the DVE blocks evenly through the sg loop so ACT/DVE interleave
DVE_SGS = tuple(g for g in range(SB)
                if (g + 1) * N_DVE_SG // SB > g * N_DVE_SG // SB)
_REST = tuple(g for g in range(SB) if g not in DVE_SGS)
GP_SGS = tuple(_REST[i] for i in range(0, 2 * N_GP_SG, 2))  # alternate
ACT_SGS = tuple(g for g in _REST if g not in GP_SGS)
N_ASG, N_DSG, N_GSG = len(ACT_SGS), len(DVE_SGS), len(GP_SGS)
# position of each sg within its path's packed mask array
_MASK_IDX = {g: i for i, g in enumerate(ACT_SGS)}
_MASK_IDX.update({g: i for i, g in enumerate(DVE_SGS)})
_MASK_IDX.update({g: i for i, g in enumerate(GP_SGS)})

_CACHE = {}


def _build_l1():
    import concourse.mybir as mybir
    import concourse.tile as tile
    from concourse import bacc

    f32 = mybir.dt.float32
    fp16 = mybir.dt.float16

    nc = bacc.Bacc("TRN2", target_bir_lowering=False, debug=False,
                   num_devices=NCORES)
    srcT = nc.dram_tensor("srcT", [D, SR], fp16, kind="ExternalInput")
    wkv = nc.dram_tensor("wkv", [D, 128], fp16, kind="ExternalInput")
    kvT = nc.dram_tensor("kvT", [128, SR], fp16, kind="ExternalOutput")

    with tile.TileContext(nc) as tc:
        with (
            tc.tile_pool(name="const", bufs=1) as constp,
            tc.tile_pool(name="big", bufs=1) as bigp,
            tc.tile_pool(name="stream", bufs=2) as streamp,
            tc.tile_pool(name="pp", bufs=1, space="PSUM") as pp,
        ):
            wkv_sb = constp.tile([128, 8 * 128], fp16)
            nc.sync.dma_start(
                out=wkv_sb.rearrange("p (j m) -> p j m", m=128),
                in_=wkv.rearrange("(j p) m -> p j m", p=128))
            sts = []
            for j in range(8):
                st = streamp.tile([128, SR], fp16, tag="xs", bufs=8,
                                  name=f"st{j}")
                eng = nc.sync if j % 2 == 0 else nc.scalar
                eng.dma_start(out=st[:], in_=srcT[j * 128:(j + 1) * 128, :])
                sts.append(st)
            kv_ps = pp.tile([128, SR], f32)
            for j in range(8):
                for q in range(4):
                    nc.tensor.matmul(
                        kv_ps[:, q * 512:(q + 1) * 512],
                        lhsT=wkv_sb[:, j * 128:(j + 1) * 128],
                        rhs=sts[j][:, q * 512:(q + 1) * 512],
                        start=(j == 0), stop=(j == 7))
            kv_sb = bigp.tile([128, SR], fp16)
            for q in range(4):
                if q % 2 == 0:
                    nc.scalar.copy(kv_sb[:, q * 512:(q + 1) * 512],
                                   kv_ps[:, q * 512:(q + 1) * 512])
                else:
                    nc.vector.tensor_copy(kv_sb[:, q * 512:(q + 1) * 512],
                                          kv_ps[:, q * 512:(q + 1) * 512])
                eng = nc.sync if q % 2 == 0 else nc.scalar
                eng.dma_start(out=kvT[:, q * 512:(q + 1) * 512],
                              in_=kv_sb[:, q * 512:(q + 1) * 512])
    nc.compile()
    return nc


def _build_l2():
    import concourse.mybir as mybir
    import concourse.tile as tile
    from concourse import bacc
    from concourse.masks import make_identity

    f32 = mybir.dt.float32
    fp16 = mybir.dt.float16
    i16 = mybir.dt.int16
    AF = mybir.ActivationFunctionType
    ALU = mybir.AluOpType

    nc = bacc.Bacc("TRN2", target_bir_lowering=False, debug=False,
                   num_devices=NCORES)
    # kt2: partitions 0-63 = kT of batches 0|1; 64-127 = batches 2|3
    kt2d = nc.dram_tensor("kt2", [128, 2 * S], fp16, kind="ExternalInput")
    # v65, batch-column order (b0, b2, b1, b3): row p, col (bb, kg', c):
    # element = v[b(bb), kg'*128 + p, c] | ones
    v65d = nc.dram_tensor("v65", [128, GK * (DQ + 1)], fp16,
                          kind="ExternalInput")
    # host-prearranged SBUF image: tgt[b][p, (half, j, t)] (contiguous rows)
    tgtd = nc.dram_tensor("tgt", [B, 128, 8 * TS], fp16,
                          kind="ExternalInput")
    # packed SBUF-image mask rows for the ACT-path src blocks (fp16)
    if N_ASG:
        maskA = nc.dram_tensor("maskA", [128, N_ASG * TS], fp16,
                               kind="ExternalInput")
    # packed Schraudolph bias image for the DVE-path src blocks (int16)
    if N_DSG:
        bmaskd = nc.dram_tensor("bmask", [128, N_DSG * TS], i16,
                                kind="ExternalInput")
    # packed exp(mask/8) image for the GPSIMD-multiply src blocks (fp16)
    if N_GSG:
        emaskd = nc.dram_tensor("emask", [128, N_GSG * TS], fp16,
                                kind="ExternalInput")
    wq = nc.dram_tensor("wq", [D, DQ], fp16, kind="ExternalInput")
    bq = nc.dram_tensor("bq", [DQ], f32, kind="ExternalInput")
    # out rows 0-63: (attn @ v)^T numerator; row 64: softmax denominator
    o = nc.dram_tensor("o", [B, DQ + 1, TS], f32, kind="ExternalOutput")

    with tile.TileContext(nc) as tc:
        with (
            tc.tile_pool(name="const", bufs=1) as constp,
            tc.tile_pool(name="big", bufs=1) as bigp,
            tc.tile_pool(name="stream", bufs=2) as streamp,
            tc.tile_pool(name="pp", bufs=1, space="PSUM") as pp,
        ):
            # ---- constants + all input DMA, issued in need-order on sync
            wq_sb = constp.tile([128, 8 * DQ], fp16)
            nc.sync.dma_start(
                out=wq_sb.rearrange("p (j m) -> p j m", m=DQ),
                in_=wq.rearrange("(j p) m -> p j m", p=128))
            bq_sb = constp.tile([128, 1], f32)
            nc.sync.dma_start(out=bq_sb[0:64, :],
                              in_=bq.rearrange("(p o) -> p o", o=1))
            nc.sync.dma_start(out=bq_sb[64:128, :],
                              in_=bq.rearrange("(p o) -> p o", o=1))
            ident = constp.tile([128, 128], fp16)
            make_identity(nc, ident[:])

            kT2 = bigp.tile([128, 2 * S], fp16)
            v2 = bigp.tile([128, GK * (DQ + 1)], fp16)
            VQ = 32 * (DQ + 1)
            if N_ASG:
                maskA_sb = bigp.tile([128, N_ASG * TS], fp16)
            if N_DSG:
                bmask_sb = bigp.tile([128, N_DSG * TS], i16)
            if N_GSG:
                emask_sb = bigp.tile([128, N_GSG * TS], fp16)
            tgs = {}
            for b in (0, 2, 1, 3):
                tgs[b] = streamp.tile([128, 8 * TS], fp16, tag="tg", bufs=4,
                                      name=f"tg{b}")

            def load_tg(b, eng):
                eng.dma_start(out=tgs[b][:], in_=tgtd[b])

            def load_kt2(lo, hi, eng):
                eng.dma_start(out=kT2[:, lo:hi], in_=kt2d[:, lo:hi])

            def load_v2(i0, eng):
                eng.dma_start(out=v2[:, i0 * VQ:(i0 + 2) * VQ],
                              in_=v65d[:, i0 * VQ:(i0 + 2) * VQ])

            def load_img(sb_tile, dram, lo, hi, n, eng):
                hi = min(hi, n)
                if n and lo < hi:
                    eng.dma_start(out=sb_tile[:, lo * TS:hi * TS],
                                  in_=dram[:, lo * TS:hi * TS])

            # need-ordered issue, split across the two HW-DGE sequencers
            load_tg(0, nc.sync)
            load_kt2(0, 2048, nc.scalar)    # column 0, sg 0-15
            load_tg(2, nc.sync)
            if N_ASG:
                load_img(maskA_sb, maskA, 0, 6, N_ASG, nc.scalar)
            if N_DSG:
                load_img(bmask_sb, bmaskd, 0, 6, N_DSG, nc.sync)
            if N_GSG:
                load_img(emask_sb, emaskd, 0, N_GSG, N_GSG, nc.scalar)
            load_v2(0, nc.sync)             # column-0 batches (b0, b2)
            if N_ASG:
                load_img(maskA_sb, maskA, 6, N_ASG, N_ASG, nc.scalar)
            if N_DSG:
                load_img(bmask_sb, bmaskd, 6, N_DSG, N_DSG, nc.sync)
            load_kt2(2048, 4096, nc.scalar)  # column 0, sg 16-31
            load_tg(1, nc.sync)
            load_tg(3, nc.sync)
            load_kt2(4096, 8192, nc.scalar)  # column 1
            load_v2(2, nc.sync)             # column-1 batches (b1, b3)

            # ---- q projection (column order; b1/b3 emitted mid-loop below)
            qT_sb = bigp.tile([128, 2 * TS], fp16)

            def qproj(b):
                pb, colb = (b // 2) * 64, (b % 2) * TS
                q_ps = pp.tile([128, TS], f32, tag="qk", bufs=3,
                               name=f"q_ps{b}")
                for j in range(8):
                    nc.tensor.matmul(
                        q_ps[pb:pb + 64, :],
                        lhsT=wq_sb[:, j * DQ:(j + 1) * DQ],
                        rhs=tgs[b][:, j * TS:(j + 1) * TS],
                        start=(j == 0), stop=(j == 7))
                nc.scalar.activation(
                    qT_sb[pb:pb + 64, colb:colb + TS], q_ps[pb:pb + 64, :],
                    AF.Identity, bias=bq_sb[pb:pb + 64, :])

            qproj(0)
            qproj(2)

            # ---- attention main loop: batch-column outer; column c handles
            # batches {c, c+2} on disjoint 64-row PE tiles.  PV emission
            # trails by PIPE_LAG so the PE never waits on an exp.
            for col in range(2):
                pv = [pp.tile([DQ + 1, TS], f32, tag=f"pv{h}",
                              name=f"pv{col}_{h}") for h in range(2)]
                pts = {}

                def emit_qk_exp(sg, col=col, pts=pts):
                    dve = sg in DVE_SGS
                    gp = sg in GP_SGS
                    mi = _MASK_IDX[sg]
                    qkt = pp.tile([128, 2 * TS], f32, tag="qk", bufs=3,
                                  name=f"qkt{col}_{sg}")
                    if not (dve or gp):
                        for half in range(2):
                            nc.tensor.matmul(
                                qkt[:, half * TS:(half + 1) * TS],
                                lhsT=ident[:],
                                rhs=maskA_sb[:, mi * TS:(mi + 1) * TS],
                                start=True, stop=False)
                    for half in range(2):
                        nc.tensor.matmul(
                            qkt[:, half * TS:(half + 1) * TS],
                            lhsT=kT2[half * 64:half * 64 + 64,
                                     col * S + sg * 128:
                                     col * S + sg * 128 + 128],
                            rhs=qT_sb[half * 64:half * 64 + 64,
                                      col * TS:(col + 1) * TS],
                            start=(dve or gp), stop=True)
                    pt = streamp.tile([128, 2 * TS], fp16, tag="P", bufs=8,
                                      name=f"pt{col}_{sg}")
                    if dve:
                        for half in range(2):
                            nc.vector.scalar_tensor_tensor(
                                out=pt[:, half * TS:(half + 1) * TS]
                                .bitcast(i16),
                                in0=qkt[:, half * TS:(half + 1) * TS],
                                scalar=float(A16),
                                in1=bmask_sb[:, mi * TS:(mi + 1) * TS],
                                op0=ALU.mult, op1=ALU.add)
                    elif gp:
                        es = streamp.tile([128, 2 * TS], fp16, tag="E",
                                          bufs=4, name=f"es{col}_{sg}")
                        nc.scalar.activation(es[:], qkt[:], AF.Exp,
                                             scale=0.125)
                        for half in range(2):
                            nc.gpsimd.tensor_tensor(
                                out=pt[:, half * TS:(half + 1) * TS],
                                in0=es[:, half * TS:(half + 1) * TS],
                                in1=emask_sb[:, mi * TS:(mi + 1) * TS],
                                op=ALU.mult)
                    else:
                        nc.scalar.activation(pt[:], qkt[:], AF.Exp,
                                             scale=0.125)
                    pts[sg] = pt

                def emit_pv(sg, col=col, pv=pv, pts=pts):
                    pt = pts.pop(sg)
                    for half in range(2):
                        kg = (2 * col + half) * SB + sg  # v65 column order
                        nc.tensor.matmul(
                            pv[half][:],
                            lhsT=v2[:, kg * (DQ + 1):(kg + 1) * (DQ + 1)],
                            rhs=pt[:, half * TS:(half + 1) * TS],
                            start=(sg == 0), stop=(sg == SB - 1))

                for sg in range(SB):
                    emit_qk_exp(sg)
                    if sg >= PIPE_LAG:
                        emit_pv(sg - PIPE_LAG)
                    if col == 0 and sg == 8:
                        qproj(1)
                    if col == 0 and sg == 10:
                        qproj(3)
                for sg in range(SB - PIPE_LAG, SB):
                    emit_pv(sg)

                for half in range(2):
                    ob = streamp.tile([DQ + 1, TS], f32, tag="ob", bufs=4,
                                      name=f"ob{col}_{half}")
                    if half == 0:
                        nc.scalar.copy(ob[:], pv[half][:])
                    else:
                        nc.vector.tensor_copy(ob[:], pv[half][:])
                    nc.sync.dma_start(out=o[col + 2 * half], in_=ob[:])
    nc.compile()
    return nc


def _get_l1():
    if "l1" not in _CACHE:
        _CACHE["l1"] = _build_l1()
    return _CACHE["l1"]


def _get_l2():
    if "l2" not in _CACHE:
        _CACHE["l2"] = _build_l2()
    return _CACHE["l2"]


def make_in_maps_l1(src, wk, wv):
    src16 = np.asarray(src).astype(FP16).reshape(B * S, D)
    wkv = np.concatenate([np.asarray(wk), np.asarray(wv)],
                         axis=1).astype(FP16)
    return [{
        "srcT": np.ascontiguousarray(src16[c * SR:(c + 1) * SR, :].T),
        "wkv": wkv,
    } for c in CORES]


def glue_l1_outputs(results):
    """Assemble kt2 / v65 from the 8 per-core kvT outputs (layout only)."""
    kvs = [np.asarray(results[c]["kvT"]) for c in CORES]
    kT_full = np.concatenate([kv[0:64] for kv in kvs], axis=1)   # [64, B*S]
    kt2 = np.ascontiguousarray(
        np.concatenate([kT_full[:, :2 * S], kT_full[:, 2 * S:]], axis=0))
    v_full = np.concatenate([kv[64:128] for kv in kvs], axis=1).T  # [B*S, 64]
    v65 = np.empty((B * S, DQ + 1), dtype=FP16)
    v65[:, :DQ] = v_full
    v65[:, DQ] = np.asarray(1.0, dtype=FP16)
    v65 = v65.reshape(B, SB, 128, DQ + 1)[[0, 2, 1, 3]]  # batch-column order
    v65 = np.ascontiguousarray(
        v65.reshape(GK, 128, DQ + 1).transpose(1, 0, 2).reshape(128, -1))
    return kt2, v65


def make_in_maps_l2(kt2, v65, tgt, mask, wq, bq, bv):
    tgt = np.asarray(tgt)
    mask = np.ascontiguousarray(mask, dtype=F32)
    wq16 = np.asarray(wq).astype(FP16)
    bq = np.ascontiguousarray(bq, dtype=F32)
    maps = []
    for c in CORES:
        # SBUF image: [b][p, (half, j, t)]  (qproj rhs chunks, contiguous)
        tgc = tgt[:, c * TS:(c + 1) * TS, :].astype(FP16)       # [B, TS, D]
        tgi = np.ascontiguousarray(
            tgc.transpose(0, 2, 1)                              # [B, D, TS]
            .reshape(B, 8, 128, TS).transpose(0, 2, 1, 3)       # [B, p, j, t]
            .reshape(B, 128, 8 * TS))
        m = {"kt2": kt2, "v65": v65, "tgt": tgi, "wq": wq16, "bq": bq}
        masknT = mask[c * TS:(c + 1) * TS, :].T  # [S, TS]: [s, t]

        def img(sgs, arr):
            # [128, n*TS] SBUF image: col g*TS+t <- arr[sgs[g]*128+p, t]
            sub = np.stack([arr[g * 128:(g + 1) * 128] for g in sgs], axis=1)
            return np.ascontiguousarray(sub.reshape(128, len(sgs) * TS))

        if N_ASG:
            m["maskA"] = img(ACT_SGS, masknT.astype(FP16))
        if N_DSG:
            m["bmask"] = img(
                DVE_SGS, np.rint(masknT * A16 + B16C).astype(np.int16))
        if N_GSG:
            m["emask"] = img(GP_SGS, np.exp(masknT * 0.125).astype(FP16))
        maps.append(m)
    return maps


def kernel(src, tgt, mask, wq, bq, wk, bk, wv, bv):
    from concourse.bass_utils import run_bass_kernel_spmd

    res1 = run_bass_kernel_spmd(_get_l1(), make_in_maps_l1(src, wk, wv),
                                core_ids=CORES)
    kt2, v65 = glue_l1_outputs(res1.results)
    res2 = run_bass_kernel_spmd(
        _get_l2(), make_in_maps_l2(kt2, v65, tgt, mask, wq, bq, bv),
        core_ids=CORES)
    bv = np.ascontiguousarray(bv, dtype=F32)
    out = np.empty((B, S, DQ), dtype=F32)
    for c in CORES:
        oc = np.asarray(res2.results[c]["o"])          # [B, 65, TS] f32
        att = oc[:, :DQ, :] / oc[:, DQ:DQ + 1, :]      # softmax division
        out[:, c * TS:(c + 1) * TS, :] = \
            att.transpose(0, 2, 1) + bv[None, None, :]
    return out


# revision 25
# speedup vs baseline: 1.6252x; 1.0073x over previous
"""Trainium2 Bass kernel for single-head cross-attention with additive mask.

Computation (matches the reference):
    q = tgt @ wq + bq
    k = src @ wk (+ bk dropped: softmax cancels a per-row constant exactly)
    v = src @ wv (bv applied on host in the epilogue)
    s = (q k^T + mask) / sqrt(DQ)
    out = softmax(s) @ v + bv

Two SPMD launches on 8 cores (all matmul inputs fp16, fp32 PSUM accum):
  L1: each core projects K and V for 1/8 of the global (B*S) src rows in a
      single fused matmul (wk|wv concatenated -> kvT [128, 2048] fp16 out).
  host: pure layout glue -- assembles kt2 (d-major K) and v65 (V with an
      appended ones column for the softmax denominator).
  L2: tgt sharded 8 ways; core c handles tgt rows [c*512,(c+1)*512) of every
      batch so its mask slice is read from HBM exactly once.

L2 computes scores transposed (src-block on PSUM partitions) batch-column
outer: column c processes batches {c, c+2} whose QK matmuls use disjoint
64-row PE tiles (tile_position) and run concurrently.  The additive mask
enters PSUM ahead of QK via an identity-weight matmul (start=True), so the
scalar engine reads (qk+mask) straight from PSUM and emits fp16 exp at
scale=1/8.  A subset of src-blocks (DVE_SGS) instead computes exp on the
otherwise-idle vector engine with a Schraudolph bit-trick in the fp16 bit
domain: bits16 = int16(qk*A16 + bmask), where bmask (host-baked int16)
carries mask*A16 + (15-sigma)*1024; the int16 tile bitcast to fp16 IS the
approximate exp (rel err ~3%, diluted to ~1.2e-2 end-to-end).  PV matmul
emission trails QK by PIPE_LAG blocks so the tensor engine never idles
waiting for an exp (keeps HAM un-throttled).  All DMA rides the two HW-DGE
engines (sync/scalar), issued in need-order so the q projection's tgt
slices land first.  PV accumulates fp32 in PSUM with a 65th "ones" output
row; the final division by the softmax denominator (+bv) runs on the host.
"""
import numpy as np

B, S, D, DQ = 4, 4096, 1024, 64
NCORES = 8
TS = S // NCORES            # 512 tgt rows per core
SR = (B * S) // NCORES      # 2048 global src rows per core (L1)
SB = S // 128               # 32 src blocks per batch
GK = B * SB                 # 128 global src blocks
CORES = list(range(NCORES))
F32 = np.float32
FP16 = np.float16
PIPE_LAG = 5

# --- per-src-block exp-path assignment ---
# S: DVE Schraudolph bit-trick (mask folded into the int16 affine bias)
# G: exact ACT exp, then GPSIMD multiply by host-baked exp(mask/8)
# V: exact ACT exp, then DVE multiply by exp(mask/8)
# M: mask into PSUM via identity matmul, then exact ACT exp
N_DVE_SG = 14
SIGMA = 0.035
A16 = (2.0 ** 10) * np.log2(np.e) / 8.0
B16C = (2.0 ** 10) * (15.0 - SIGMA)
# spread the Schraudolph blocks evenly; interleave the rest by type
DVE_SGS = tuple(g for g in range(SB)
                if (g + 1) * N_DVE_SG // SB > g * N_DVE_SG // SB)
_REST = tuple(g for g in range(SB) if g not in DVE_SGS)
GP_SGS = tuple(_REST[i] for i in range(0, 18, 2))            # 9 blocks
V2_SGS = tuple(_REST[i] for i in range(1, 18, 4))            # 5 blocks
ACT_SGS = tuple(g for g in _REST if g not in GP_SGS and g not in V2_SGS)
EM_SGS = tuple(sorted(GP_SGS + V2_SGS))                      # share emask
N_ASG, N_DSG, N_GSG = len(ACT_SGS), len(DVE_SGS), len(EM_SGS)
# position of each sg within its path's packed mask array
_MASK_IDX = {g: i for i, g in enumerate(ACT_SGS)}
_MASK_IDX.update({g: i for i, g in enumerate(DVE_SGS)})
_MASK_IDX.update({g: i for i, g in enumerate(EM_SGS)})

_CACHE = {}


def _build_l1():
    import concourse.mybir as mybir
    import concourse.tile as tile
    from concourse import bacc

    f32 = mybir.dt.float32
    fp16 = mybir.dt.float16

    nc = bacc.Bacc("TRN2", target_bir_lowering=False, debug=False,
                   num_devices=NCORES)
    srcT = nc.dram_tensor("srcT", [D, SR], fp16, kind="ExternalInput")
    wkv = nc.dram_tensor("wkv", [D, 128], fp16, kind="ExternalInput")
    kvT = nc.dram_tensor("kvT", [128, SR], fp16, kind="ExternalOutput")

    with tile.TileContext(nc) as tc:
        with (
            tc.tile_pool(name="const", bufs=1) as constp,
            tc.tile_pool(name="big", bufs=1) as bigp,
            tc.tile_pool(name="stream", bufs=2) as streamp,
            tc.tile_pool(name="pp", bufs=1, space="PSUM") as pp,
        ):
            wkv_sb = constp.tile([128, 8 * 128], fp16)
            nc.sync.dma_start(
                out=wkv_sb.rearrange("p (j m) -> p j m", m=128),
                in_=wkv.rearrange("(j p) m -> p j m", p=128))
            sts = []
            for j in range(8):
                st = streamp.tile([128, SR], fp16, tag="xs", bufs=8,
                                  name=f"st{j}")
                eng = nc.sync if j % 2 == 0 else nc.scalar
                eng.dma_start(out=st[:], in_=srcT[j * 128:(j + 1) * 128, :])
                sts.append(st)
            kv_ps = pp.tile([128, SR], f32)
            for j in range(8):
                for q in range(4):
                    nc.tensor.matmul(
                        kv_ps[:, q * 512:(q + 1) * 512],
                        lhsT=wkv_sb[:, j * 128:(j + 1) * 128],
                        rhs=sts[j][:, q * 512:(q + 1) * 512],
                        start=(j == 0), stop=(j == 7))
            kv_sb = bigp.tile([128, SR], fp16)
            for q in range(4):
                if q % 2 == 0:
                    nc.scalar.copy(kv_sb[:, q * 512:(q + 1) * 512],
                                   kv_ps[:, q * 512:(q + 1) * 512])
                else:
                    nc.vector.tensor_copy(kv_sb[:, q * 512:(q + 1) * 512],
                                          kv_ps[:, q * 512:(q + 1) * 512])
                eng = nc.sync if q % 2 == 0 else nc.scalar
                eng.dma_start(out=kvT[:, q * 512:(q + 1) * 512],
                              in_=kv_sb[:, q * 512:(q + 1) * 512])
    nc.compile()
    return nc


def _build_l2():
    import concourse.mybir as mybir
    import concourse.tile as tile
    from concourse import bacc
    from concourse.masks import make_identity

    f32 = mybir.dt.float32
    fp16 = mybir.dt.float16
    i16 = mybir.dt.int16
    AF = mybir.ActivationFunctionType
    ALU = mybir.AluOpType

    nc = bacc.Bacc("TRN2", target_bir_lowering=False, debug=False,
                   num_devices=NCORES)
    # kt2: partitions 0-63 = kT of batches 0|1; 64-127 = batches 2|3
    kt2d = nc.dram_tensor("kt2", [128, 2 * S], fp16, kind="ExternalInput")
    # v65, batch-column order (b0, b2, b1, b3): row p, col (bb, kg', c):
    # element = v[b(bb), kg'*128 + p, c] | ones
    v65d = nc.dram_tensor("v65", [128, GK * (DQ + 1)], fp16,
                          kind="ExternalInput")
    # host-prearranged SBUF image: tgt[b][p, (half, j, t)] (contiguous rows)
    tgtd = nc.dram_tensor("tgt", [B, 128, 8 * TS], fp16,
                          kind="ExternalInput")
    # packed SBUF-image mask rows for the ACT-path src blocks (fp16)
    if N_ASG:
        maskA = nc.dram_tensor("maskA", [128, N_ASG * TS], fp16,
                               kind="ExternalInput")
    # packed Schraudolph bias image for the DVE-path src blocks (int16)
    if N_DSG:
        bmaskd = nc.dram_tensor("bmask", [128, N_DSG * TS], i16,
                                kind="ExternalInput")
    # packed exp(mask/8) image for the GPSIMD-multiply src blocks (fp16)
    if N_GSG:
        emaskd = nc.dram_tensor("emask", [128, N_GSG * TS], fp16,
                                kind="ExternalInput")
    wq = nc.dram_tensor("wq", [D, DQ], fp16, kind="ExternalInput")
    bq = nc.dram_tensor("bq", [DQ], f32, kind="ExternalInput")
    # out rows 0-63: (attn @ v)^T numerator; row 64: softmax denominator
    o = nc.dram_tensor("o", [B, DQ + 1, TS], f32, kind="ExternalOutput")

    with tile.TileContext(nc) as tc:
        with (
            tc.tile_pool(name="const", bufs=1) as constp,
            tc.tile_pool(name="big", bufs=1) as bigp,
            tc.tile_pool(name="stream", bufs=2) as streamp,
            tc.tile_pool(name="pp", bufs=1, space="PSUM") as pp,
        ):
            # ---- constants + all input DMA, issued in need-order on sync
            wq_sb = constp.tile([128, 8 * DQ], fp16)
            nc.sync.dma_start(
                out=wq_sb.rearrange("p (j m) -> p j m", m=DQ),
                in_=wq.rearrange("(j p) m -> p j m", p=128))
            bq_sb = constp.tile([128, 1], f32)
            nc.sync.dma_start(out=bq_sb[0:64, :],
                              in_=bq.rearrange("(p o) -> p o", o=1))
            nc.sync.dma_start(out=bq_sb[64:128, :],
                              in_=bq.rearrange("(p o) -> p o", o=1))
            ident = constp.tile([128, 128], fp16)
            make_identity(nc, ident[:])

            kT2 = bigp.tile([128, 2 * S], fp16)
            v2 = bigp.tile([128, GK * (DQ + 1)], fp16)
            VQ = 32 * (DQ + 1)
            if N_ASG:
                maskA_sb = bigp.tile([128, N_ASG * TS], fp16)
            if N_DSG:
                bmask_sb = bigp.tile([128, N_DSG * TS], i16)
            if N_GSG:
                emask_sb = bigp.tile([128, N_GSG * TS], fp16)
            tgs = {}
            for b in (0, 2, 1, 3):
                tgs[b] = streamp.tile([128, 8 * TS], fp16, tag="tg", bufs=4,
                                      name=f"tg{b}")

            def load_tg(b, eng):
                eng.dma_start(out=tgs[b][:], in_=tgtd[b])

            def load_kt2(lo, hi, eng):
                eng.dma_start(out=kT2[:, lo:hi], in_=kt2d[:, lo:hi])

            def load_v2(i0, eng):
                eng.dma_start(out=v2[:, i0 * VQ:(i0 + 2) * VQ],
                              in_=v65d[:, i0 * VQ:(i0 + 2) * VQ])

            def load_img(sb_tile, dram, lo, hi, n, eng):
                hi = min(hi, n)
                if n and lo < hi:
                    eng.dma_start(out=sb_tile[:, lo * TS:hi * TS],
                                  in_=dram[:, lo * TS:hi * TS])

            # need-ordered issue, split across the two HW-DGE sequencers;
            # tiny first chunks so sg 0 unblocks as early as possible
            load_tg(0, nc.sync)
            load_kt2(0, 512, nc.scalar)     # column 0, sg 0-3
            if N_DSG:
                load_img(bmask_sb, bmaskd, 0, 3, N_DSG, nc.scalar)
            if N_GSG:
                load_img(emask_sb, emaskd, 0, 3, N_GSG, nc.scalar)
            load_tg(2, nc.sync)
            if N_ASG:
                load_img(maskA_sb, maskA, 0, N_ASG, N_ASG, nc.scalar)
            load_kt2(512, 2048, nc.scalar)  # column 0, sg 4-15
            if N_DSG:
                load_img(bmask_sb, bmaskd, 3, 8, N_DSG, nc.sync)
            if N_GSG:
                load_img(emask_sb, emaskd, 3, 8, N_GSG, nc.scalar)
            load_v2(0, nc.sync)             # column-0 batches (b0, b2)
            if N_DSG:
                load_img(bmask_sb, bmaskd, 8, N_DSG, N_DSG, nc.sync)
            if N_GSG:
                load_img(emask_sb, emaskd, 8, N_GSG, N_GSG, nc.scalar)
            load_kt2(2048, 4096, nc.scalar)  # column 0, sg 16-31
            load_tg(1, nc.sync)
            load_tg(3, nc.sync)
            load_kt2(4096, 8192, nc.scalar)  # column 1
            load_v2(2, nc.sync)             # column-1 batches (b1, b3)

            # ---- q projection (column order; b1/b3 emitted mid-loop below)
            qT_sb = bigp.tile([128, 2 * TS], fp16)

            def qproj(b):
                pb, colb = (b // 2) * 64, (b % 2) * TS
                q_ps = pp.tile([128, TS], f32, tag="qk", bufs=3,
                               name=f"q_ps{b}")
                for j in range(8):
                    nc.tensor.matmul(
                        q_ps[pb:pb + 64, :],
                        lhsT=wq_sb[:, j * DQ:(j + 1) * DQ],
                        rhs=tgs[b][:, j * TS:(j + 1) * TS],
                        start=(j == 0), stop=(j == 7))
                nc.scalar.activation(
                    qT_sb[pb:pb + 64, colb:colb + TS], q_ps[pb:pb + 64, :],
                    AF.Identity, bias=bq_sb[pb:pb + 64, :])

            qproj(0)
            qproj(2)

            # ---- attention main loop: batch-column outer; column c handles
            # batches {c, c+2} on disjoint 64-row PE tiles.  PV emission
            # trails by PIPE_LAG so the PE never waits on an exp.
            for col in range(2):
                pv = [pp.tile([DQ + 1, TS], f32, tag=f"pv{h}",
                              name=f"pv{col}_{h}") for h in range(2)]
                pts = {}

                def emit_qk_exp(sg, col=col, pts=pts):
                    dve = sg in DVE_SGS
                    em = sg in EM_SGS
                    mi = _MASK_IDX[sg]
                    qkt = pp.tile([128, 2 * TS], f32, tag="qk", bufs=3,
                                  name=f"qkt{col}_{sg}")
                    if not (dve or em):
                        for half in range(2):
                            nc.tensor.matmul(
                                qkt[:, half * TS:(half + 1) * TS],
                                lhsT=ident[:],
                                rhs=maskA_sb[:, mi * TS:(mi + 1) * TS],
                                start=True, stop=False)
                    for half in range(2):
                        nc.tensor.matmul(
                            qkt[:, half * TS:(half + 1) * TS],
                            lhsT=kT2[half * 64:half * 64 + 64,
                                     col * S + sg * 128:
                                     col * S + sg * 128 + 128],
                            rhs=qT_sb[half * 64:half * 64 + 64,
                                      col * TS:(col + 1) * TS],
                            start=(dve or em), stop=True)
                    pt = streamp.tile([128, 2 * TS], fp16, tag="P", bufs=8,
                                      name=f"pt{col}_{sg}")
                    if dve:
                        for half in range(2):
                            nc.vector.scalar_tensor_tensor(
                                out=pt[:, half * TS:(half + 1) * TS]
                                .bitcast(i16),
                                in0=qkt[:, half * TS:(half + 1) * TS],
                                scalar=float(A16),
                                in1=bmask_sb[:, mi * TS:(mi + 1) * TS],
                                op0=ALU.mult, op1=ALU.add)
                    elif em:
                        es = streamp.tile([128, 2 * TS], fp16, tag="E",
                                          bufs=4, name=f"es{col}_{sg}")
                        nc.scalar.activation(es[:], qkt[:], AF.Exp,
                                             scale=0.125)
                        eng = nc.gpsimd if sg in GP_SGS else nc.vector
                        for half in range(2):
                            eng.tensor_tensor(
                                out=pt[:, half * TS:(half + 1) * TS],
                                in0=es[:, half * TS:(half + 1) * TS],
                                in1=emask_sb[:, mi * TS:(mi + 1) * TS],
                                op=ALU.mult)
                    else:
                        nc.scalar.activation(pt[:], qkt[:], AF.Exp,
                                             scale=0.125)
                    pts[sg] = pt

                def emit_pv(sg, col=col, pv=pv, pts=pts):
                    pt = pts.pop(sg)
                    for half in range(2):
                        kg = (2 * col + half) * SB + sg  # v65 column order
                        nc.tensor.matmul(
                            pv[half][:],
                            lhsT=v2[:, kg * (DQ + 1):(kg + 1) * (DQ + 1)],
                            rhs=pt[:, half * TS:(half + 1) * TS],
                            start=(sg == 0), stop=(sg == SB - 1))

                for sg in range(SB):
                    emit_qk_exp(sg)
                    if sg >= PIPE_LAG:
                        emit_pv(sg - PIPE_LAG)
                    if col == 0 and sg == 8:
                        qproj(1)
                    if col == 0 and sg == 10:
                        qproj(3)
                for sg in range(SB - PIPE_LAG, SB):
                    emit_pv(sg)

                for half in range(2):
                    ob = streamp.tile([DQ + 1, TS], f32, tag="ob", bufs=4,
                                      name=f"ob{col}_{half}")
                    if half == 0:
                        nc.scalar.copy(ob[:], pv[half][:])
                    else:
                        nc.vector.tensor_copy(ob[:], pv[half][:])
                    nc.sync.dma_start(out=o[col + 2 * half], in_=ob[:])
    nc.compile()
    return nc


def _get_l1():
    if "l1" not in _CACHE:
        _CACHE["l1"] = _build_l1()
    return _CACHE["l1"]


def _get_l2():
    if "l2" not in _CACHE:
        _CACHE["l2"] = _build_l2()
    return _CACHE["l2"]


def make_in_maps_l1(src, wk, wv):
    src16 = np.asarray(src).astype(FP16).reshape(B * S, D)
    wkv = np.concatenate([np.asarray(wk), np.asarray(wv)],
                         axis=1).astype(FP16)
    return [{
        "srcT": np.ascontiguousarray(src16[c * SR:(c + 1) * SR, :].T),
        "wkv": wkv,
    } for c in CORES]


def glue_l1_outputs(results):
    """Assemble kt2 / v65 from the 8 per-core kvT outputs (layout only)."""
    kvs = [np.asarray(results[c]["kvT"]) for c in CORES]
    kT_full = np.concatenate([kv[0:64] for kv in kvs], axis=1)   # [64, B*S]
    kt2 = np.ascontiguousarray(
        np.concatenate([kT_full[:, :2 * S], kT_full[:, 2 * S:]], axis=0))
    v_full = np.concatenate([kv[64:128] for kv in kvs], axis=1).T  # [B*S, 64]
    v65 = np.empty((B * S, DQ + 1), dtype=FP16)
    v65[:, :DQ] = v_full
    v65[:, DQ] = np.asarray(1.0, dtype=FP16)
    v65 = v65.reshape(B, SB, 128, DQ + 1)[[0, 2, 1, 3]]  # batch-column order
    v65 = np.ascontiguousarray(
        v65.reshape(GK, 128, DQ + 1).transpose(1, 0, 2).reshape(128, -1))
    return kt2, v65


def make_in_maps_l2(kt2, v65, tgt, mask, wq, bq, bv):
    tgt = np.asarray(tgt)
    mask = np.ascontiguousarray(mask, dtype=F32)
    wq16 = np.asarray(wq).astype(FP16)
    bq = np.ascontiguousarray(bq, dtype=F32)
    maps = []
    for c in CORES:
        # SBUF image: [b][p, (half, j, t)]  (qproj rhs chunks, contiguous)
        tgc = tgt[:, c * TS:(c + 1) * TS, :].astype(FP16)       # [B, TS, D]
        tgi = np.ascontiguousarray(
            tgc.transpose(0, 2, 1)                              # [B, D, TS]
            .reshape(B, 8, 128, TS).transpose(0, 2, 1, 3)       # [B, p, j, t]
            .reshape(B, 128, 8 * TS))
        m = {"kt2": kt2, "v65": v65, "tgt": tgi, "wq": wq16, "bq": bq}
        masknT = mask[c * TS:(c + 1) * TS, :].T  # [S, TS]: [s, t]

        def img(sgs, arr):
            # [128, n*TS] SBUF image: col g*TS+t <- arr[sgs[g]*128+p, t]
            sub = np.stack([arr[g * 128:(g + 1) * 128] for g in sgs], axis=1)
            return np.ascontiguousarray(sub.reshape(128, len(sgs) * TS))

        if N_ASG:
            m["maskA"] = img(ACT_SGS, masknT.astype(FP16))
        if N_DSG:
            m["bmask"] = img(
                DVE_SGS, np.rint(masknT * A16 + B16C).astype(np.int16))
        if N_GSG:
            m["emask"] = img(EM_SGS, np.exp(masknT * 0.125).astype(FP16))
        maps.append(m)
    return maps


def kernel(src, tgt, mask, wq, bq, wk, bk, wv, bv):
    from concourse.bass_utils import run_bass_kernel_spmd

    res1 = run_bass_kernel_spmd(_get_l1(), make_in_maps_l1(src, wk, wv),
                                core_ids=CORES)
    kt2, v65 = glue_l1_outputs(res1.results)
    res2 = run_bass_kernel_spmd(
        _get_l2(), make_in_maps_l2(kt2, v65, tgt, mask, wq, bq, bv),
        core_ids=CORES)
    bv = np.ascontiguousarray(bv, dtype=F32)
    out = np.empty((B, S, DQ), dtype=F32)
    for c in CORES:
        oc = np.asarray(res2.results[c]["o"])          # [B, 65, TS] f32
        att = oc[:, :DQ, :] / oc[:, DQ:DQ + 1, :]      # softmax division
        out[:, c * TS:(c + 1) * TS, :] = \
            att.transpose(0, 2, 1) + bv[None, None, :]
    return out
